# revision 27
# baseline (speedup 1.0000x reference)
"""CaptchaCRNN Trainium2 kernel: 7 convs + 2 train-mode BN + maxpools + biLSTM.

Data-parallel over batch on 8 NeuronCores (8 images/core). BN batch stats are
globalized with a tiny AllReduce. Conv matmuls run in float32r (1 cyc/row).
"""
import sys

sys.path.insert(0, "/opt/trn_rl_repo")

import numpy as np
import concourse.bass as bass
import concourse.bacc as bacc
import concourse.tile as tile
from concourse import mybir
from concourse import bass_utils

F32 = mybir.dt.float32
F32R = mybir.dt.float32r
AF = mybir.ActivationFunctionType
ALU = mybir.AluOpType
AX = mybir.AxisListType

NCORES = 8
B = 8          # images per core
EPS = 1e-5
INV_N = 1.0 / (64 * 8 * 32)   # BN normalizer: full batch 64 x H8 x W32

# 4H gate permutation: torch order [i,f,g,o] -> compute order [i,f,o,g]
PERM4H = np.r_[0:512, 768:1024, 512:768]


def _ap(obj, offset, dims):
    base = obj if isinstance(obj, bass.AP) else obj[:]
    return bass.AP(tensor=base.tensor, offset=base.offset + offset,
                   ap=[list(d) for d in dims])


def build(debug=False):
    nc = bacc.Bacc("TRN2", target_bir_lowering=False, debug=False,
                   enable_asserts=True, num_devices=NCORES)

    def din(name, shape):
        return nc.dram_tensor(name, list(shape), F32, kind="ExternalInput").ap()

    def dout(name, shape):
        return nc.dram_tensor(name, list(shape), F32, kind="ExternalOutput").ap()

    xpad = din("xpad", (B, 66, 258))
    w1T = din("w1T", (9, 64))
    b1 = din("b1", (64, 1))
    w2p = din("w2p", (3, 128, 128))
    w2s = din("w2s", (3, 64, 128))
    w3T = din("w3T", (1, 9, 128, 256))
    w4T = din("w4T", (2, 9, 128, 256))
    w5T = din("w5T", (2, 9, 128, 512))
    w6T = din("w6T", (4, 9, 128, 512))
    w7T = din("w7T", (4, 4, 128, 512))
    b2 = din("b2", (128, 1))
    b3 = din("b3", (128, 2))
    b4 = din("b4", (128, 2))
    b5 = din("b5", (128, 4))
    b6 = din("b6", (128, 4))
    b7 = din("b7", (128, 4))
    gam = din("gam", (128, 4))
    bet = din("bet", (128, 4))
    wihT = din("wihT", (2, 8, 128, 1024))
    whhT = din("whhT", (2, 2, 128, 1024))
    lbias = din("lbias", (128, 2, 8))
    out = dout("out", (B, 15, 512))

    dbg = {}
    if debug:
        dbg["a2"] = dout("dbg_a2", (128, 8, 16, 64))
        dbg["a4"] = dout("dbg_a4", (128, 2, 8, 8, 32))
        dbg["a5"] = dout("dbg_a5", (128, 4, 8, 8, 32))
        dbg["c6p"] = dout("dbg_c6p", (128, 4, 8, 4, 16))
        dbg["c7"] = dout("dbg_c7", (128, 4, 8, 3, 16))
        dbg["xg"] = dout("dbg_xg", (128, 2, 8, 8, 15))
        dbg["hs"] = dout("dbg_hs", (128, 2, 2, 8, 15))

    with tile.TileContext(nc) as tc:
        opened = []

        def popen(name, bufs, space="SBUF", side=None):
            cm = tc.tile_pool(name=name, bufs=bufs, space=space, side=side)
            p = cm.__enter__()
            p._cm = cm
            opened.append(p)
            return p

        def pclose(p):
            p._cm.__exit__(None, None, None)
            opened.remove(p)

        const = popen("const", 1, side="left")
        psum = popen("psum", 8, space="PSUM")
        dram = popen("dram", 1, space="DRAM")

        # ---- constants ----
        b1_sb = const.tile([64, 1], F32)
        nc.sync.dma_start(out=b1_sb[:], in_=b1)
        b2_sb = const.tile([128, 1], F32)
        nc.sync.dma_start(out=b2_sb[:], in_=b2)
        b3_sb = const.tile([128, 2], F32)
        nc.sync.dma_start(out=b3_sb[:], in_=b3)
        b4_sb = const.tile([128, 2], F32)
        nc.sync.dma_start(out=b4_sb[:], in_=b4)
        b5_sb = const.tile([128, 4], F32)
        nc.sync.dma_start(out=b5_sb[:], in_=b5)
        b6_sb = const.tile([128, 4], F32)
        nc.sync.dma_start(out=b6_sb[:], in_=b6)
        b7_sb = const.tile([128, 4], F32)
        nc.sync.dma_start(out=b7_sb[:], in_=b7)
        gam_sb = const.tile([128, 4], F32)
        nc.sync.dma_start(out=gam_sb[:], in_=gam)
        bet_sb = const.tile([128, 4], F32)
        nc.sync.dma_start(out=bet_sb[:], in_=bet)
        eps_sb = const.tile([128, 1], F32)
        nc.vector.memset(eps_sb[:], EPS)

        # ---- conv1 weights ----
        wA = popen("wA", 1, side="left")
        w1_sb = wA.tile([9, 64], F32R)
        nc.sync.dma_start(out=w1_sb[:], in_=w1T.bitcast(F32R))
        w2p_sb = wA.tile([128, 3, 128], F32R)
        # w2p dram (3,128,128): partition = axis1
        nc.sync.dma_start(
            out=w2p_sb[:],
            in_=_ap(w2p, 0, [[128, 128], [128 * 128, 3], [1, 128]]).bitcast(F32R))
        w2s_sb = wA.tile([128, 3, 128], F32R)
        nc.sync.dma_start(
            out=w2s_sb[64:128, :, :],
            in_=_ap(w2s, 0, [[128, 64], [64 * 128, 3], [1, 128]]).bitcast(F32R))
        w3_sb = wA.tile([128, 9, 256], F32R)
        nc.sync.dma_start(
            out=w3_sb[:],
            in_=_ap(w3T, 0, [[256, 128], [128 * 256, 9], [1, 256]]).bitcast(F32R))
        w4_sb = wA.tile([128, 2, 9, 256], F32R)
        nc.sync.dma_start(
            out=w4_sb[:],
            in_=_ap(w4T, 0, [[256, 128], [9 * 128 * 256, 2], [128 * 256, 9],
                             [1, 256]]).bitcast(F32R))

        # ---- conv1 + pool1 -> p1d DRAM (B, 64, 32, 128) ----
        p1d = dram.tile([B, 64, 32, 128], F32)
        rhs1 = popen("rhs1", 2, side="left")
        st1 = popen("st1", 2, side="left")
        for b in range(B):
            for q in range(4):
                r0 = 16 * q
                rt = rhs1.tile([9, 16, 256], F32R, tag="rhs1")
                off = b * (66 * 258) + r0 * 258
                for kh in range(3):
                    nc.sync.dma_start(
                        out=rt[3 * kh:3 * kh + 3, :, :],
                        in_=_ap(xpad, off + kh * 258,
                                [[1, 3], [258, 16], [1, 256]]).bitcast(F32R))
                s1q = st1.tile([64, 16, 256], F32, tag="s1q")
                rv = rt[:].rearrange("k (j t) w -> k j (t w)", t=2)
                for j in range(8):
                    pt = psum.tile([64, 512], F32, tag="ps")
                    nc.tensor.matmul(pt[:], w1_sb[:], rv[:, j, :],
                                     start=True, stop=True)
                    nc.scalar.activation(
                        out=s1q[:, 2 * j:2 * j + 2, :].rearrange("k a b -> k (a b)"),
                        in_=pt[:], func=AF.Identity, bias=b1_sb[:, 0:1], scale=1.0)
                wq = st1.tile([64, 16, 128], F32, tag="wq")
                s1v = s1q[:].rearrange("k h (w two) -> k h w two", two=2)
                nc.vector.tensor_max(out=wq[:], in0=s1v[:, :, :, 0],
                                     in1=s1v[:, :, :, 1])
                hq = st1.tile([64, 8, 128], F32, tag="hq")
                wv = wq[:].rearrange("k (h two) w -> k h two w", two=2)
                nc.vector.tensor_max(out=hq[:], in0=wv[:, :, 0, :],
                                     in1=wv[:, :, 1, :])
                nc.sync.dma_start(
                    out=_ap(p1d, b * (64 * 32 * 128) + (8 * q) * 128,
                            [[32 * 128, 64], [128, 8], [1, 128]]),
                    in_=hq[:])
        pclose(st1)
        pclose(rhs1)

        # ---- conv2 + pool2 -> a2p [128, B, 18, 66] ----
        a2p_pool = popen("a2p", 1, side="right")
        a2p = a2p_pool.tile([128, B, 18, 66], F32R)
        # zero borders of a2p once
        for r in (0, 17):
            nc.gpsimd.memset(a2p[:, :, r, :].bitcast(F32), 0.0)
        for c in (0, 65):
            nc.gpsimd.memset(a2p[:, :, :, c].bitcast(F32), 0.0)

        a1b_pool = popen("a1b", 2, side="left")
        st2 = popen("st2", 3, side="left")
        for b in range(B):
            a1b = a1b_pool.tile([128, 34, 130], F32R, tag="a1b")
            for r in (0, 33):
                nc.gpsimd.memset(a1b[0:64, r, :].bitcast(F32), 0.0)
            nc.gpsimd.memset(a1b[64:128, 32:34, :].bitcast(F32), 0.0)
            for c in (0, 129):
                nc.gpsimd.memset(a1b[:, :, c].bitcast(F32), 0.0)
            src = _ap(p1d, b * (64 * 32 * 128),
                      [[32 * 128, 64], [128, 32], [1, 128]]).bitcast(F32R)
            nc.sync.dma_start(out=a1b[0:64, 1:33, 1:129], in_=src)
            nc.sync.dma_start(out=a1b[64:128, 0:32, 1:129], in_=src)
            for n in range(8):
                h0 = 4 * n
                pt = psum.tile([128, 4, 128], F32, tag="ps")
                for kw in range(3):
                    nc.tensor.matmul(
                        pt[:], w2p_sb[:, kw, :],
                        a1b[0:128, h0:h0 + 4, kw:kw + 128],
                        start=(kw == 0), stop=False)
                for kw in range(3):
                    nc.tensor.matmul(
                        pt[:], w2s_sb[64:128, kw, :],
                        a1b[64:128, h0 + 1:h0 + 5, kw:kw + 128],
                        start=False, stop=(kw == 2))
                s2 = st2.tile([128, 4, 128], F32, tag="s2")
                nc.scalar.copy(out=s2[:].rearrange("p a b -> p (a b)"), in_=pt[:])
                w2m = st2.tile([128, 4, 64], F32, tag="w2m")
                s2v = s2[:].rearrange("p h (w two) -> p h w two", two=2)
                nc.vector.tensor_max(out=w2m[:], in0=s2v[:, :, :, 0],
                                     in1=s2v[:, :, :, 1])
                wv = w2m[:].rearrange("p (h two) w -> p h two w", two=2)
                nc.vector.tensor_max(out=a2p[:, b, 1 + 2 * n:3 + 2 * n, 1:65],
                                     in0=wv[:, :, 0, :], in1=wv[:, :, 1, :])
        # bias b2 on interior
        nc.scalar.activation(out=a2p[:, :, 1:17, 1:65], in_=a2p[:, :, 1:17, 1:65].bitcast(F32),
                             func=AF.Identity, bias=b2_sb[:, 0:1], scale=1.0)
        pclose(st2)
        pclose(a1b_pool)
        if debug:
            for b in range(B):
                nc.sync.dma_start(
                    out=_ap(dbg["a2"], b * 1024, [[8192, 128], [64, 16], [1, 64]]),
                    in_=a2p[:, b, 1:17, 1:65].bitcast(F32))

        # ---- conv3 -> a3 [128, 2, B, 18, 66] ----
        a3_pool = popen("a3", 1, side="left")
        a3 = a3_pool.tile([128, 2, B, 18, 66], F32R)
        for g in range(2):
            for r in (0, 17):
                nc.gpsimd.memset(a3[:, g, :, r, :].bitcast(F32), 0.0)
            for c in (0, 65):
                nc.gpsimd.memset(a3[:, g, :, :, c].bitcast(F32), 0.0)
        for b in range(B):
            for m in range(2):
                for n in range(2):
                    pt = psum.tile([128, 8, 64], F32, tag="ps")
                    for kh in range(3):
                        for kw in range(3):
                            tap = kh * 3 + kw
                            nc.tensor.matmul(
                                pt[:], w3_sb[:, tap, 128 * m:128 * m + 128],
                                a2p[:, b, 8 * n + kh:8 * n + kh + 8, kw:kw + 64],
                                start=(tap == 0), stop=(tap == 8))
                    nc.scalar.activation(
                        out=a3[:, m, b, 1 + 8 * n:9 + 8 * n, 1:65],
                        in_=pt[:], func=AF.Identity, bias=b3_sb[:, m:m + 1],
                        scale=1.0)
        pclose(a2p_pool)

        # ---- conv4 + pool4 -> a4p [128, 2, B, 10, 34] ----
        a4p_pool = popen("a4p", 1, side="right")
        a4p = a4p_pool.tile([128, 2, B, 10, 34], F32R)
        for g in range(2):
            for r in (0, 9):
                nc.gpsimd.memset(a4p[:, g, :, r, :].bitcast(F32), 0.0)
            for c in (0, 33):
                nc.gpsimd.memset(a4p[:, g, :, :, c].bitcast(F32), 0.0)
        # prefetch w5
        wB = popen("wB", 1, side="right")
        w5_sb = wB.tile([128, 2, 9, 512], F32R)
        nc.gpsimd.dma_start(
            out=w5_sb[:],
            in_=_ap(w5T, 0, [[512, 128], [9 * 128 * 512, 2], [128 * 512, 9],
                             [1, 512]]).bitcast(F32R))
        st4 = popen("st4", 3, side="right")
        for b in range(B):
            for m in range(2):
                for n in range(2):
                    pt = psum.tile([128, 8, 64], F32, tag="ps")
                    for g in range(2):
                        for kh in range(3):
                            for kw in range(3):
                                i = g * 9 + kh * 3 + kw
                                nc.tensor.matmul(
                                    pt[:], w4_sb[:, g, kh * 3 + kw,
                                                 128 * m:128 * m + 128],
                                    a3[:, g, b, 8 * n + kh:8 * n + kh + 8,
                                       kw:kw + 64],
                                    start=(i == 0), stop=(i == 17))
                    s4 = st4.tile([128, 8, 64], F32, tag="s4")
                    nc.scalar.copy(out=s4[:].rearrange("p a b -> p (a b)"),
                                   in_=pt[:])
                    s4v = s4[:].rearrange("p (h two) (w v) -> p h two w v",
                                          two=2, v=2)
                    nc.vector.tensor_max(
                        out=a4p[:, m, b, 1 + 4 * n:5 + 4 * n, 1:33],
                        in0=s4v[:, :, 0, :, 0], in1=s4v[:, :, 0, :, 1])
        for m in range(2):
            nc.scalar.activation(out=a4p[:, m, :, 1:9, 1:33],
                                 in_=a4p[:, m, :, 1:9, 1:33].bitcast(F32),
                                 func=AF.Identity, bias=b4_sb[:, m:m + 1],
                                 scale=1.0)
        pclose(st4)
        pclose(a3_pool)
        pclose(wA)
        if debug:
            for m in range(2):
                for b in range(B):
                    nc.sync.dma_start(
                        out=_ap(dbg["a4"], m * 2048 + b * 256,
                                [[4096, 128], [32, 8], [1, 32]]),
                        in_=a4p[:, m, b, 1:9, 1:33].bitcast(F32))

        # ---- conv5 -> c5 [128, 4, B, 8, 32]; BN5 -> a5 [128, 4, B, 10, 34] ----
        stat_pool = popen("stat", 1, side="left")
        scr_pool = popen("scr", 2, side="left")
        c5_pool = popen("c5", 1, side="left")
        c5 = c5_pool.tile([128, 4, B, 8, 32], F32)
        for bp in range(4):
            b0 = 2 * bp
            for m in range(4):
                pt = psum.tile([128, 2, 8, 32], F32, tag="ps")
                for g in range(2):
                    for kh in range(3):
                        for kw in range(3):
                            i = g * 9 + kh * 3 + kw
                            nc.tensor.matmul(
                                pt[:], w5_sb[:, g, kh * 3 + kw,
                                             128 * m:128 * m + 128],
                                a4p[:, g, b0:b0 + 2, kh:kh + 8, kw:kw + 32],
                                start=(i == 0), stop=(i == 17))
                nc.scalar.activation(
                    out=c5[:, m, b0:b0 + 2, :, :], in_=pt[:],
                    func=AF.Identity, bias=b5_sb[:, m:m + 1], scale=1.0)
        pclose(wB)
        pclose(a4p_pool)

        # BN5 stats
        st5 = stat_pool.tile([128, 8], F32)
        nc.vector.tensor_reduce(
            out=st5[:, 0:4], in_=c5[:].rearrange("p g b h w -> p g (b h w)"),
            axis=AX.X, op=ALU.add)
        for m in range(4):
            scr = scr_pool.tile([128, 2048], F32, tag="scr")
            nc.scalar.activation(
                out=scr[:], in_=c5[:, m].rearrange("p b h w -> p (b h w)"),
                func=AF.Square, bias=0.0, scale=1.0,
                accum_out=st5[:, 4 + m:5 + m])
        cc5i = dram.tile([128, 8], F32, tag="cc5i")
        cc5o = dram.tile([128, 8], F32, tag="cc5o")
        nc.gpsimd.dma_start(out=cc5i[:], in_=st5[:])
        nc.gpsimd.collective_compute(
            "AllReduce", ALU.add, replica_groups=[list(range(NCORES))],
            ins=[cc5i[:].opt()], outs=[cc5o[:].opt()])
        g5 = stat_pool.tile([128, 8], F32)
        nc.sync.dma_start(out=g5[:], in_=cc5o[:])
        mean5 = stat_pool.tile([128, 4], F32)
        nc.scalar.activation(out=mean5[:], in_=g5[:, 0:4], func=AF.Copy,
                             bias=0.0, scale=INV_N)
        ex25 = stat_pool.tile([128, 4], F32)
        nc.scalar.activation(out=ex25[:], in_=g5[:, 4:8], func=AF.Copy,
                             bias=0.0, scale=INV_N)
        var5 = stat_pool.tile([128, 4], F32)
        nc.vector.tensor_mul(out=var5[:], in0=mean5[:], in1=mean5[:])
        nc.vector.tensor_sub(out=var5[:], in0=ex25[:], in1=var5[:])
        std5 = stat_pool.tile([128, 4], F32)
        nc.scalar.activation(out=std5[:], in_=var5[:], func=AF.Sqrt,
                             bias=eps_sb[:, 0:1], scale=1.0)
        nc.vector.reciprocal(out=std5[:], in_=std5[:])
        aa5 = stat_pool.tile([128, 4], F32)
        nc.vector.tensor_mul(out=aa5[:], in0=std5[:], in1=gam_sb[:])
        dd5 = stat_pool.tile([128, 4], F32)
        nc.vector.tensor_mul(out=dd5[:], in0=mean5[:], in1=aa5[:])
        nc.vector.tensor_sub(out=dd5[:], in0=bet_sb[:], in1=dd5[:])

        a5_pool = popen("a5", 1, side="right")
        a5 = a5_pool.tile([128, 4, B, 10, 34], F32R)
        for g in range(4):
            for r in (0, 9):
                nc.gpsimd.memset(a5[:, g, :, r, :].bitcast(F32), 0.0)
            for c in (0, 33):
                nc.gpsimd.memset(a5[:, g, :, :, c].bitcast(F32), 0.0)
        for m in range(4):
            nc.scalar.activation(
                out=a5[:, m, :, 1:9, 1:33], in_=c5[:, m],
                func=AF.Identity, bias=dd5[:, m:m + 1], scale=aa5[:, m:m + 1])
        pclose(c5_pool)
        if debug:
            for m in range(4):
                for b in range(B):
                    nc.sync.dma_start(
                        out=_ap(dbg["a5"], m * 2048 + b * 256,
                                [[8192, 128], [32, 8], [1, 32]]),
                        in_=a5[:, m, b, 1:9, 1:33].bitcast(F32))

        # ---- conv6 -> c6; stats6; pool6 -> c6p; BN6 on pooled ----
        wC = popen("wC", 2, side="left")
        c6_pool = popen("c6", 1, side="left")
        c6 = c6_pool.tile([128, 4, B, 8, 32], F32)
        for wave in range(2):
            bps = [2 * wave, 2 * wave + 1]
            pts = {}
            for m in range(4):
                for bp in bps:
                    pts[(m, bp)] = psum.tile([128, 2, 8, 32], F32, tag="ps",
                                             name=f"ps6_{m}_{bp}")
            for g in range(4):
                w6g = wC.tile([128, 9, 512], F32R, tag="w6g")
                nc.gpsimd.dma_start(
                    out=w6g[:],
                    in_=_ap(w6T, g * (9 * 128 * 512),
                            [[512, 128], [128 * 512, 9], [1, 512]]).bitcast(F32R))
                for tap in range(9):
                    kh, kw = tap // 3, tap % 3
                    for m in range(4):
                        for bp in bps:
                            b0 = 2 * bp
                            i = g * 9 + tap
                            nc.tensor.matmul(
                                pts[(m, bp)][:],
                                w6g[:, tap, 128 * m:128 * m + 128],
                                a5[:, g, b0:b0 + 2, kh:kh + 8, kw:kw + 32],
                                start=(i == 0), stop=(i == 35))
            for m in range(4):
                for bp in bps:
                    b0 = 2 * bp
                    nc.scalar.activation(
                        out=c6[:, m, b0:b0 + 2, :, :], in_=pts[(m, bp)][:],
                        func=AF.Identity, bias=b6_sb[:, m:m + 1], scale=1.0)
        pclose(a5_pool)

        # stats6 + pool6 in parallel
        st6 = stat_pool.tile([128, 8], F32)
        nc.vector.tensor_reduce(
            out=st6[:, 0:4], in_=c6[:].rearrange("p g b h w -> p g (b h w)"),
            axis=AX.X, op=ALU.add)
        for m in range(4):
            scr = scr_pool.tile([128, 2048], F32, tag="scr")
            nc.scalar.activation(
                out=scr[:], in_=c6[:, m].rearrange("p b h w -> p (b h w)"),
                func=AF.Square, bias=0.0, scale=1.0,
                accum_out=st6[:, 4 + m:5 + m])
        cc6i = dram.tile([128, 8], F32, tag="cc6i")
        cc6o = dram.tile([128, 8], F32, tag="cc6o")
        nc.gpsimd.dma_start(out=cc6i[:], in_=st6[:])
        nc.gpsimd.collective_compute(
            "AllReduce", ALU.add, replica_groups=[list(range(NCORES))],
            ins=[cc6i[:].opt()], outs=[cc6o[:].opt()])

        c6p_pool = popen("c6p", 1, side="right")
        c6p = c6p_pool.tile([128, 4, B, 4, 17], F32R)
        for g in range(4):
            nc.gpsimd.memset(c6p[:, g, :, :, 16].bitcast(F32), 0.0)
        for m in range(4):
            c6v = c6[:, m].rearrange("p b (h two) (w v) -> p b h two w v",
                                     two=2, v=2)
            nc.vector.tensor_max(out=c6p[:, m, :, :, 0:16], in0=c6v[:, :, :, 0, :, 0],
                                 in1=c6v[:, :, :, 0, :, 1])

        g6 = stat_pool.tile([128, 8], F32)
        nc.sync.dma_start(out=g6[:], in_=cc6o[:])
        mean6 = stat_pool.tile([128, 4], F32)
        nc.scalar.activation(out=mean6[:], in_=g6[:, 0:4], func=AF.Copy,
                             bias=0.0, scale=INV_N)
        ex26 = stat_pool.tile([128, 4], F32)
        nc.scalar.activation(out=ex26[:], in_=g6[:, 4:8], func=AF.Copy,
                             bias=0.0, scale=INV_N)
        var6 = stat_pool.tile([128, 4], F32)
        nc.vector.tensor_mul(out=var6[:], in0=mean6[:], in1=mean6[:])
        nc.vector.tensor_sub(out=var6[:], in0=ex26[:], in1=var6[:])
        std6 = stat_pool.tile([128, 4], F32)
        nc.scalar.activation(out=std6[:], in_=var6[:], func=AF.Sqrt,
                             bias=eps_sb[:, 0:1], scale=1.0)
        nc.vector.reciprocal(out=std6[:], in_=std6[:])
        aa6 = stat_pool.tile([128, 4], F32)
        nc.vector.tensor_mul(out=aa6[:], in0=std6[:], in1=gam_sb[:])
        dd6 = stat_pool.tile([128, 4], F32)
        nc.vector.tensor_mul(out=dd6[:], in0=mean6[:], in1=aa6[:])
        nc.vector.tensor_sub(out=dd6[:], in0=bet_sb[:], in1=dd6[:])
        for m in range(4):
            nc.scalar.activation(
                out=c6p[:, m], in_=c6p[:, m].bitcast(F32),
                func=AF.Identity, bias=dd6[:, m:m + 1], scale=aa6[:, m:m + 1])
        pclose(c6_pool)
        pclose(wC)
        pclose(scr_pool)
        pclose(stat_pool)
        if debug:
            for g in range(4):
                for b in range(B):
                    nc.sync.dma_start(
                        out=_ap(dbg["c6p"], g * 512 + b * 64,
                                [[2048, 128], [16, 4], [1, 16]]),
                        in_=c6p[:, g, b, :, 0:16].bitcast(F32))

        # ---- conv7 (VALID 2x2) -> c7 [128, 4, B, 3, 15] ----
        wD = popen("wD", 1, side="right")
        w7_sb = wD.tile([128, 4, 4, 512], F32R)
        nc.gpsimd.dma_start(
            out=w7_sb[:],
            in_=_ap(w7T, 0, [[512, 128], [4 * 128 * 512, 4], [128 * 512, 4],
                             [1, 512]]).bitcast(F32R))
        c7_pool = popen("c7", 1, side="left")
        c7 = c7_pool.tile([128, 4, B, 3, 16], F32)
        for m in range(4):
            pt = psum.tile([128, 8, 3, 16], F32, tag="ps")
            for g in range(4):
                for tap in range(4):
                    kh, kw = tap // 2, tap % 2
                    i = g * 4 + tap
                    nc.tensor.matmul(
                        pt[:], w7_sb[:, g, tap, 128 * m:128 * m + 128],
                        c6p[:, g, :, kh:kh + 3, kw:kw + 16],
                        start=(i == 0), stop=(i == 15))
            nc.scalar.activation(out=c7[:, m], in_=pt[:], func=AF.Identity,
                                 bias=b7_sb[:, m:m + 1], scale=1.0)
        pclose(wD)
        pclose(c6p_pool)
        if debug:
            nc.sync.dma_start(
                out=_ap(dbg["c7"], 0, [[1536, 128], [1, 1536]]),
                in_=c7[:].rearrange("p g b h w -> p (g b h w)"))

        # ---- LSTM ----
        wE = popen("wE", 1, side="right")
        wih_sb = wE.tile([128, 2, 8, 1024], F32)
        nc.gpsimd.dma_start(
            out=wih_sb[:],
            in_=_ap(wihT, 0, [[1024, 128], [8 * 128 * 1024, 2], [128 * 1024, 8],
                              [1, 1024]]))
        whh_sb = wE.tile([128, 2, 2, 1024], F32)
        nc.gpsimd.dma_start(
            out=whh_sb[:],
            in_=_ap(whhT, 0, [[1024, 128], [2 * 128 * 1024, 2], [128 * 1024, 2],
                              [1, 1024]]))
        lb_sb = wE.tile([128, 2, 8], F32)
        nc.sync.dma_start(out=lb_sb[:], in_=lbias)

        ls = popen("ls", 1, side="right")
        xg = ls.tile([128, 2, 8, 8, 15], F32)
        hs = ls.tile([128, 2, 2, 8, 15], F32)
        cst = ls.tile([128, 2, 2, 8], F32)
        gp = popen("gp", 4, side="right")
        tp = popen("tp", 4, side="right")

        for dr in range(2):
            for m in range(8):
                pt = psum.tile([128, 8, 16], F32, tag="ps")
                for gd in range(8):
                    nc.tensor.matmul(
                        pt[:], wih_sb[:, dr, gd, 128 * m:128 * m + 128],
                        c7[:, gd % 4, :, gd // 4, :],
                        start=(gd == 0), stop=(gd == 7))
                nc.scalar.activation(
                    out=xg[:, dr, m],
                    in_=pt[:, :, 0:15],
                    func=AF.Identity, bias=lb_sb[:, dr, m:m + 1], scale=1.0)
        if debug:
            nc.sync.dma_start(
                out=_ap(dbg["xg"], 0, [[1920, 128], [1, 1920]]),
                in_=xg[:].rearrange("p d m b t -> p (d m b t)"))

        for t in range(15):
            for dr in range(2):
                tt = t if dr == 0 else 14 - t
                ga = gp.tile([128, 8, 8], F32, tag="ga")
                if t == 0:
                    nc.vector.tensor_copy(out=ga[:], in_=xg[:, dr, :, :, tt])
                else:
                    tprev = tt - 1 if dr == 0 else tt + 1
                    pr = psum.tile([128, 8, 8], F32, tag="ps")
                    for m in range(8):
                        for gh in range(2):
                            nc.tensor.matmul(
                                pr[:, m, :],
                                whh_sb[:, dr, gh, 128 * m:128 * m + 128],
                                hs[:, dr, gh, :, tprev],
                                start=(gh == 0), stop=(gh == 1),
                                skip_group_check=True)
                    nc.vector.tensor_add(out=ga[:], in0=pr[:],
                                         in1=xg[:, dr, :, :, tt])
                nc.scalar.activation(out=ga[:, 0:6, :], in_=ga[:, 0:6, :],
                                     func=AF.Sigmoid, bias=0.0, scale=1.0)
                nc.scalar.activation(out=ga[:, 6:8, :], in_=ga[:, 6:8, :],
                                     func=AF.Tanh, bias=0.0, scale=1.0)
                cs = cst[:, dr]
                if t == 0:
                    nc.vector.tensor_mul(out=cs, in0=ga[:, 0:2, :],
                                         in1=ga[:, 6:8, :])
                else:
                    t1 = tp.tile([128, 2, 8], F32, tag="t1")
                    nc.vector.tensor_mul(out=t1[:], in0=ga[:, 0:2, :],
                                         in1=ga[:, 6:8, :])
                    t2 = tp.tile([128, 2, 8], F32, tag="t2")
                    nc.vector.tensor_mul(out=t2[:], in0=ga[:, 2:4, :], in1=cs)
                    nc.vector.tensor_add(out=cs, in0=t1[:], in1=t2[:])
                th = tp.tile([128, 2, 8], F32, tag="th")
                nc.scalar.activation(out=th[:], in_=cs, func=AF.Tanh,
                                     bias=0.0, scale=1.0)
                nc.vector.tensor_mul(out=hs[:, dr, :, :, tt],
                                     in0=ga[:, 4:6, :], in1=th[:])

        if debug:
            nc.sync.dma_start(
                out=_ap(dbg["hs"], 0, [[480, 128], [1, 480]]),
                in_=hs[:].rearrange("p d g b t -> p (d g b t)"))

        # ---- output DMA: out[b, t, 256*dr + 128*gh + p] ----
        for dr in range(2):
            for gh in range(2):
                nc.sync.dma_start(
                    out=_ap(out, 256 * dr + 128 * gh,
                            [[1, 128], [512, 120]]),
                    in_=hs[:, dr, gh].rearrange("p b t -> p (b t)"))

        for p in reversed(list(opened)):
            pclose(p)

    nc.compile()
    return nc


def prep_inputs(inputs, core):
    """Host-side: shard + transform weights for one core."""
    d = {}
    x = np.asarray(inputs["x"], dtype=np.float32)
    xs = x[core * B:(core + 1) * B, 0]          # (8, 64, 256)
    xpad = np.zeros((B, 66, 258), np.float32)
    xpad[:, 1:65, 1:257] = xs
    d["xpad"] = xpad

    w1 = np.asarray(inputs["w1"], np.float32)   # (64,1,3,3)
    d["w1T"] = np.ascontiguousarray(
        w1[:, 0].reshape(64, 9).T)              # (9, 64)
    d["b1"] = np.asarray(inputs["b1"], np.float32).reshape(64, 1)

    w2 = np.asarray(inputs["w2"], np.float32)   # (128,64,3,3)
    w2p = np.zeros((3, 128, 128), np.float32)
    for kw in range(3):
        w2p[kw, 0:64] = w2[:, :, 0, kw].T
        w2p[kw, 64:128] = w2[:, :, 1, kw].T
    d["w2p"] = w2p
    d["w2s"] = np.ascontiguousarray(
        np.transpose(w2[:, :, 2, :], (2, 1, 0)))  # (3, 64, 128)
    d["b2"] = np.asarray(inputs["b2"], np.float32).reshape(1, 128).T.copy()

    def wT(w, gK, cout):
        # w (O, I, 3, 3) -> (gK, 9, 128, O)
        o, i_, kh, kw = w.shape
        r = np.transpose(w, (2, 3, 1, 0)).reshape(kh * kw, gK, 128, o)
        return np.ascontiguousarray(np.transpose(r, (1, 0, 2, 3)))

    d["w3T"] = wT(np.asarray(inputs["w3"], np.float32), 1, 256)
    d["w4T"] = wT(np.asarray(inputs["w4"], np.float32), 2, 256)
    d["w5T"] = wT(np.asarray(inputs["w5"], np.float32), 2, 512)
    d["w6T"] = wT(np.asarray(inputs["w6"], np.float32), 4, 512)
    w7 = np.asarray(inputs["w7"], np.float32)   # (512,512,2,2)
    r7 = np.transpose(w7, (2, 3, 1, 0)).reshape(4, 4, 128, 512)
    d["w7T"] = np.ascontiguousarray(np.transpose(r7, (1, 0, 2, 3)))
    for k, g in (("b3", 2), ("b4", 2), ("b5", 4), ("b6", 4), ("b7", 4)):
        src = "b" + k[1]
        d[k] = np.ascontiguousarray(
            np.asarray(inputs[src], np.float32).reshape(g, 128).T)
    d["gam"] = np.ascontiguousarray(
        np.asarray(inputs["gamma"], np.float32).reshape(4, 128).T)
    d["bet"] = np.ascontiguousarray(
        np.asarray(inputs["beta"], np.float32).reshape(4, 128).T)

    # LSTM: d-column permutation dmap maps compute-chunk col 128*j+p to
    # reference D index 2*(128*(j%4)+p) + j//4
    j = np.arange(8)[:, None]
    p = np.arange(128)[None, :]
    dmap = (2 * (128 * (j % 4) + p) + j // 4).reshape(-1)
    wih = np.stack([np.asarray(inputs["Wih_f"], np.float32),
                    np.asarray(inputs["Wih_b"], np.float32)])
    whh = np.stack([np.asarray(inputs["Whh_f"], np.float32),
                    np.asarray(inputs["Whh_b"], np.float32)])
    wihp = wih[:, PERM4H][:, :, dmap]           # (2, 1024, 1024)
    d["wihT"] = np.ascontiguousarray(
        np.transpose(wihp, (0, 2, 1)).reshape(2, 8, 128, 1024))
    whhp = whh[:, PERM4H]                       # (2, 1024, 256)
    d["whhT"] = np.ascontiguousarray(
        np.transpose(whhp, (0, 2, 1)).reshape(2, 2, 128, 1024))
    lb = (np.stack([np.asarray(inputs["bih_f"], np.float32),
                    np.asarray(inputs["bih_b"], np.float32)])
          + np.stack([np.asarray(inputs["bhh_f"], np.float32),
                      np.asarray(inputs["bhh_b"], np.float32)]))
    lbp = lb[:, PERM4H].reshape(2, 8, 128)      # (dir, m, p)
    d["lbias"] = np.ascontiguousarray(np.transpose(lbp, (2, 0, 1)))
    return d


_NC_CACHE = {}


def kernel(**inputs):
    key = "debug" if inputs.pop("_debug", False) else "main"
    if key not in _NC_CACHE:
        _NC_CACHE[key] = build(debug=(key == "debug"))
    nc = _NC_CACHE[key]
    in_maps = [prep_inputs(inputs, c) for c in range(NCORES)]
    res = bass_utils.run_bass_kernel_spmd(nc, in_maps,
                                          core_ids=list(range(NCORES)))
    out = np.concatenate([res.results[c]["out"] for c in range(NCORES)], axis=0)
    kernel.last_results = res
    return out


# revision 30
# speedup vs baseline: 1.3061x; 1.3061x over previous
"""CaptchaCRNN Trainium2 kernel: 7 convs + 2 train-mode BN + maxpools + biLSTM.

Data-parallel over batch on 8 NeuronCores (8 images/core). BN batch stats are
globalized with a tiny AllReduce. Conv matmuls run in float32r (1 cyc/row).
"""
import sys

sys.path.insert(0, "/opt/trn_rl_repo")

import numpy as np
import concourse.bass as bass
import concourse.bacc as bacc
import concourse.tile as tile
from concourse import mybir
from concourse import bass_utils

F32 = mybir.dt.float32
F16 = mybir.dt.float16
F32R = mybir.dt.float32r
AF = mybir.ActivationFunctionType
ALU = mybir.AluOpType
AX = mybir.AxisListType

NCORES = 8
B = 8          # images per core
EPS = 1e-5
INV_N = 1.0 / (64 * 8 * 32)   # BN normalizer: full batch 64 x H8 x W32

# 4H gate permutation: torch order [i,f,g,o] -> compute order [i,f,o,g]
PERM4H = np.r_[0:512, 768:1024, 512:768]


def _ap(obj, offset, dims):
    base = obj if isinstance(obj, bass.AP) else obj[:]
    return bass.AP(tensor=base.tensor, offset=base.offset + offset,
                   ap=[list(d) for d in dims])


def build(debug=False):
    nc = bacc.Bacc("TRN2", target_bir_lowering=False, debug=False,
                   enable_asserts=True, num_devices=NCORES)

    def din(name, shape):
        return nc.dram_tensor(name, list(shape), F32, kind="ExternalInput").ap()

    def dout(name, shape):
        return nc.dram_tensor(name, list(shape), F32, kind="ExternalOutput").ap()

    xim = din("xim", (B, 9, 64, 256))
    w1T = din("w1T", (9, 64))
    b1 = din("b1", (64, 1))
    w2p = din("w2p", (3, 128, 128))
    w2s = din("w2s", (3, 64, 128))
    w3T = din("w3T", (1, 9, 128, 256))
    w4T = din("w4T", (2, 9, 128, 256))
    w5T = din("w5T", (2, 9, 128, 512))
    w6T = din("w6T", (4, 9, 128, 512))
    w7T = din("w7T", (4, 4, 128, 512))
    b2 = din("b2", (128, 1))
    b3 = din("b3", (128, 2))
    b4 = din("b4", (128, 2))
    b5 = din("b5", (128, 4))
    b6 = din("b6", (128, 4))
    b7 = din("b7", (128, 4))
    gam = din("gam", (128, 4))
    bet = din("bet", (128, 4))
    wihT = din("wihT", (2, 8, 128, 1024))
    whhT = nc.dram_tensor("whhT", [2, 2, 128, 1024], mybir.dt.float16,
                          kind="ExternalInput").ap()
    lbias = din("lbias", (128, 2, 8))
    out = dout("out", (B, 15, 512))

    dbg = {}
    if debug:
        dbg["a2"] = dout("dbg_a2", (128, 8, 16, 64))
        dbg["a4"] = dout("dbg_a4", (128, 2, 8, 8, 32))
        dbg["a5"] = dout("dbg_a5", (128, 4, 8, 8, 32))
        dbg["c6p"] = dout("dbg_c6p", (128, 4, 8, 4, 16))
        dbg["c7"] = dout("dbg_c7", (128, 4, 8, 3, 16))
        dbg["xg"] = dout("dbg_xg", (128, 2, 8, 8, 15))
        dbg["hs"] = dout("dbg_hs", (128, 2, 2, 8, 15))

    with tile.TileContext(nc) as tc:
        opened = []

        def popen(name, bufs, space="SBUF", side=None):
            cm = tc.tile_pool(name=name, bufs=bufs, space=space, side=side)
            p = cm.__enter__()
            p._cm = cm
            opened.append(p)
            return p

        def pclose(p):
            p._cm.__exit__(None, None, None)
            opened.remove(p)

        const = popen("const", 1, side="left")
        psum = popen("psum", 8, space="PSUM")
        dram = popen("dram", 1, space="DRAM")

        # ---- constants ----
        b1_sb = const.tile([64, 1], F32)
        nc.sync.dma_start(out=b1_sb[:], in_=b1)
        b2_sb = const.tile([128, 1], F32)
        nc.sync.dma_start(out=b2_sb[:], in_=b2)
        b3_sb = const.tile([128, 2], F32)
        nc.sync.dma_start(out=b3_sb[:], in_=b3)
        b4_sb = const.tile([128, 2], F32)
        nc.sync.dma_start(out=b4_sb[:], in_=b4)
        b5_sb = const.tile([128, 4], F32)
        nc.sync.dma_start(out=b5_sb[:], in_=b5)
        b6_sb = const.tile([128, 4], F32)
        nc.sync.dma_start(out=b6_sb[:], in_=b6)
        b7_sb = const.tile([128, 4], F32)
        nc.sync.dma_start(out=b7_sb[:], in_=b7)
        gam_sb = const.tile([128, 4], F32)
        nc.sync.dma_start(out=gam_sb[:], in_=gam)
        bet_sb = const.tile([128, 4], F32)
        nc.sync.dma_start(out=bet_sb[:], in_=bet)
        eps_sb = const.tile([128, 1], F32)
        nc.vector.memset(eps_sb[:], EPS)

        # ---- conv1 weights ----
        wA = popen("wA", 1, side="left")
        w1_sb = wA.tile([9, 64], F32R)
        nc.sync.dma_start(out=w1_sb[:], in_=w1T.bitcast(F32R))
        w2p_sb = wA.tile([128, 3, 128], F32R)
        # w2p dram (3,128,128): partition = axis1
        nc.sync.dma_start(
            out=w2p_sb[:],
            in_=_ap(w2p, 0, [[128, 128], [128 * 128, 3], [1, 128]]).bitcast(F32R))
        w2s_sb = wA.tile([128, 3, 128], F32R)
        nc.sync.dma_start(
            out=w2s_sb[64:128, :, :],
            in_=_ap(w2s, 0, [[128, 64], [64 * 128, 3], [1, 128]]).bitcast(F32R))
        w3_sb = wA.tile([128, 9, 256], F32R)
        nc.sync.dma_start(
            out=w3_sb[:],
            in_=_ap(w3T, 0, [[256, 128], [128 * 256, 9], [1, 256]]).bitcast(F32R))
        w4_sb = wA.tile([128, 2, 9, 256], F32R)
        nc.sync.dma_start(
            out=w4_sb[:],
            in_=_ap(w4T, 0, [[256, 128], [9 * 128 * 256, 2], [128 * 256, 9],
                             [1, 256]]).bitcast(F32R))

        # ---- conv1 + pool1 -> p1d DRAM (B, 64, 32, 128) ----
        p1d = dram.tile([B, 64, 32, 128], F32)
        rhs1 = popen("rhs1", 2, side="left")
        st1 = popen("st1", 2, side="left")
        for b in range(B):
            for q in range(4):
                r0 = 16 * q
                rt = rhs1.tile([9, 16, 256], F32R, tag="rhs1")
                nc.sync.dma_start(
                    out=rt[:],
                    in_=_ap(xim, b * (9 * 64 * 256) + r0 * 256,
                            [[64 * 256, 9], [1, 16 * 256]]).bitcast(F32R))
                s1q = st1.tile([64, 16, 256], F32, tag="s1q")
                rv = rt[:].rearrange("k (j t) w -> k j (t w)", t=2)
                for j in range(8):
                    pt = psum.tile([64, 512], F32, tag="ps")
                    nc.tensor.matmul(pt[:], w1_sb[:], rv[:, j, :],
                                     start=True, stop=True)
                    nc.scalar.activation(
                        out=s1q[:, 2 * j:2 * j + 2, :].rearrange("k a b -> k (a b)"),
                        in_=pt[:], func=AF.Identity, bias=b1_sb[:, 0:1], scale=1.0)
                wq = st1.tile([64, 16, 128], F32, tag="wq")
                s1v = s1q[:].rearrange("k h (w two) -> k h w two", two=2)
                nc.vector.tensor_max(out=wq[:], in0=s1v[:, :, :, 0],
                                     in1=s1v[:, :, :, 1])
                hq = st1.tile([64, 8, 128], F32, tag="hq")
                wv = wq[:].rearrange("k (h two) w -> k h two w", two=2)
                nc.vector.tensor_max(out=hq[:], in0=wv[:, :, 0, :],
                                     in1=wv[:, :, 1, :])
                nc.sync.dma_start(
                    out=_ap(p1d, b * (64 * 32 * 128) + (8 * q) * 128,
                            [[32 * 128, 64], [128, 8], [1, 128]]),
                    in_=hq[:])
        pclose(st1)
        pclose(rhs1)

        # ---- conv2 + pool2 -> a2p [128, B, 18, 66] ----
        a2p_pool = popen("a2p", 1, side="right")
        a2p = a2p_pool.tile([128, B, 18, 66], F32R)
        # zero borders of a2p once
        for r in (0, 17):
            nc.gpsimd.memset(a2p[:, :, r, :].bitcast(F32), 0.0)
        for c in (0, 65):
            nc.gpsimd.memset(a2p[:, :, :, c].bitcast(F32), 0.0)

        a1b_pool = popen("a1b", 2, side="left")
        st2 = popen("st2", 3, side="left")
        for b in range(B):
            a1b = a1b_pool.tile([128, 34, 130], F32R, tag="a1b")
            for r in (0, 33):
                nc.gpsimd.memset(a1b[0:64, r, :].bitcast(F32), 0.0)
            nc.gpsimd.memset(a1b[64:128, 32:34, :].bitcast(F32), 0.0)
            for c in (0, 129):
                nc.gpsimd.memset(a1b[:, :, c].bitcast(F32), 0.0)
            src = _ap(p1d, b * (64 * 32 * 128),
                      [[32 * 128, 64], [128, 32], [1, 128]]).bitcast(F32R)
            nc.sync.dma_start(out=a1b[0:64, 1:33, 1:129], in_=src)
            nc.sync.dma_start(out=a1b[64:128, 0:32, 1:129], in_=src)
            for n in range(8):
                h0 = 4 * n
                pt = psum.tile([128, 4, 128], F32, tag="ps")
                for kw in range(3):
                    nc.tensor.matmul(
                        pt[:], w2p_sb[:, kw, :],
                        a1b[0:128, h0:h0 + 4, kw:kw + 128],
                        start=(kw == 0), stop=False)
                for kw in range(3):
                    nc.tensor.matmul(
                        pt[:], w2s_sb[64:128, kw, :],
                        a1b[64:128, h0 + 1:h0 + 5, kw:kw + 128],
                        start=False, stop=(kw == 2))
                s2 = st2.tile([128, 4, 128], F32, tag="s2")
                nc.scalar.copy(out=s2[:].rearrange("p a b -> p (a b)"), in_=pt[:])
                w2m = st2.tile([128, 4, 64], F32, tag="w2m")
                s2v = s2[:].rearrange("p h (w two) -> p h w two", two=2)
                nc.vector.tensor_max(out=w2m[:], in0=s2v[:, :, :, 0],
                                     in1=s2v[:, :, :, 1])
                wv = w2m[:].rearrange("p (h two) w -> p h two w", two=2)
                nc.vector.tensor_max(out=a2p[:, b, 1 + 2 * n:3 + 2 * n, 1:65],
                                     in0=wv[:, :, 0, :], in1=wv[:, :, 1, :])
        # bias b2 on interior
        nc.scalar.activation(out=a2p[:, :, 1:17, 1:65], in_=a2p[:, :, 1:17, 1:65].bitcast(F32),
                             func=AF.Identity, bias=b2_sb[:, 0:1], scale=1.0)
        pclose(st2)
        pclose(a1b_pool)
        if debug:
            for b in range(B):
                nc.sync.dma_start(
                    out=_ap(dbg["a2"], b * 1024, [[8192, 128], [64, 16], [1, 64]]),
                    in_=a2p[:, b, 1:17, 1:65].bitcast(F32))

        # ---- conv3 -> a3 [128, 2, B, 18, 66] ----
        a3_pool = popen("a3", 1, side="left")
        a3 = a3_pool.tile([128, 2, B, 18, 66], F32R)
        for g in range(2):
            for r in (0, 17):
                nc.gpsimd.memset(a3[:, g, :, r, :].bitcast(F32), 0.0)
            for c in (0, 65):
                nc.gpsimd.memset(a3[:, g, :, :, c].bitcast(F32), 0.0)
        for b in range(B):
            for m in range(2):
                for n in range(2):
                    pt = psum.tile([128, 8, 64], F32, tag="ps")
                    for kh in range(3):
                        for kw in range(3):
                            tap = kh * 3 + kw
                            nc.tensor.matmul(
                                pt[:], w3_sb[:, tap, 128 * m:128 * m + 128],
                                a2p[:, b, 8 * n + kh:8 * n + kh + 8, kw:kw + 64],
                                start=(tap == 0), stop=(tap == 8))
                    nc.scalar.activation(
                        out=a3[:, m, b, 1 + 8 * n:9 + 8 * n, 1:65],
                        in_=pt[:], func=AF.Identity, bias=b3_sb[:, m:m + 1],
                        scale=1.0)
        pclose(a2p_pool)

        # ---- conv4 + pool4 -> a4p [128, 2, B, 10, 34] ----
        a4p_pool = popen("a4p", 1, side="right")
        a4p = a4p_pool.tile([128, 2, B, 10, 34], F32R)
        for g in range(2):
            for r in (0, 9):
                nc.gpsimd.memset(a4p[:, g, :, r, :].bitcast(F32), 0.0)
            for c in (0, 33):
                nc.gpsimd.memset(a4p[:, g, :, :, c].bitcast(F32), 0.0)
        # prefetch w5
        wB = popen("wB", 1, side="right")
        w5_sb = wB.tile([128, 2, 9, 512], F32R)
        nc.gpsimd.dma_start(
            out=w5_sb[:],
            in_=_ap(w5T, 0, [[512, 128], [9 * 128 * 512, 2], [128 * 512, 9],
                             [1, 512]]).bitcast(F32R))
        st4 = popen("st4", 3, side="right")
        for b in range(B):
            for m in range(2):
                for n in range(2):
                    pt = psum.tile([128, 8, 64], F32, tag="ps")
                    for g in range(2):
                        for kh in range(3):
                            for kw in range(3):
                                i = g * 9 + kh * 3 + kw
                                nc.tensor.matmul(
                                    pt[:], w4_sb[:, g, kh * 3 + kw,
                                                 128 * m:128 * m + 128],
                                    a3[:, g, b, 8 * n + kh:8 * n + kh + 8,
                                       kw:kw + 64],
                                    start=(i == 0), stop=(i == 17))
                    s4 = st4.tile([128, 8, 64], F32, tag="s4")
                    nc.scalar.copy(out=s4[:].rearrange("p a b -> p (a b)"),
                                   in_=pt[:])
                    s4v = s4[:].rearrange("p (h two) (w v) -> p h two w v",
                                          two=2, v=2)
                    nc.vector.tensor_max(
                        out=a4p[:, m, b, 1 + 4 * n:5 + 4 * n, 1:33],
                        in0=s4v[:, :, 0, :, 0], in1=s4v[:, :, 0, :, 1])
        for m in range(2):
            nc.scalar.activation(out=a4p[:, m, :, 1:9, 1:33],
                                 in_=a4p[:, m, :, 1:9, 1:33].bitcast(F32),
                                 func=AF.Identity, bias=b4_sb[:, m:m + 1],
                                 scale=1.0)
        pclose(st4)
        pclose(a3_pool)
        pclose(wA)
        if debug:
            for m in range(2):
                for b in range(B):
                    nc.sync.dma_start(
                        out=_ap(dbg["a4"], m * 2048 + b * 256,
                                [[4096, 128], [32, 8], [1, 32]]),
                        in_=a4p[:, m, b, 1:9, 1:33].bitcast(F32))

        # ---- conv5 -> c5 [128, 4, B, 8, 32]; BN5 -> a5 [128, 4, B, 10, 34] ----
        wC = popen("wC", 2, side="left")
        w6seq = [(wv, g) for wv in range(2) for g in range(4)]
        w6tiles = {}

        def load_w6(i):
            wv, g = w6seq[i]
            t = wC.tile([128, 9, 512], F32R, tag="w6g", name=f"w6g_{wv}_{g}")
            nc.gpsimd.dma_start(
                out=t[:],
                in_=_ap(w6T, g * (9 * 128 * 512),
                        [[512, 128], [128 * 512, 9], [1, 512]]).bitcast(F32R))
            w6tiles[(wv, g)] = t

        load_w6(0)
        load_w6(1)

        stat_pool = popen("stat", 1, side="left")
        scr_pool = popen("scr", 2, side="left")
        c5_pool = popen("c5", 1, side="left")
        c5 = c5_pool.tile([128, 4, B, 8, 32], F32)
        for bp in range(4):
            b0 = 2 * bp
            for m in range(4):
                pt = psum.tile([128, 2, 8, 32], F32, tag="ps")
                for g in range(2):
                    for kh in range(3):
                        for kw in range(3):
                            i = g * 9 + kh * 3 + kw
                            nc.tensor.matmul(
                                pt[:], w5_sb[:, g, kh * 3 + kw,
                                             128 * m:128 * m + 128],
                                a4p[:, g, b0:b0 + 2, kh:kh + 8, kw:kw + 32],
                                start=(i == 0), stop=(i == 17))
                nc.scalar.activation(
                    out=c5[:, m, b0:b0 + 2, :, :], in_=pt[:],
                    func=AF.Identity, bias=b5_sb[:, m:m + 1], scale=1.0)
        pclose(wB)
        pclose(a4p_pool)

        # BN5 stats
        st5 = stat_pool.tile([128, 8], F32)
        nc.vector.tensor_reduce(
            out=st5[:, 0:4], in_=c5[:].rearrange("p g b h w -> p g (b h w)"),
            axis=AX.X, op=ALU.add)
        for m in range(4):
            scr = scr_pool.tile([128, 2048], F32, tag="scr")
            nc.scalar.activation(
                out=scr[:], in_=c5[:, m].rearrange("p b h w -> p (b h w)"),
                func=AF.Square, bias=0.0, scale=1.0,
                accum_out=st5[:, 4 + m:5 + m])
        cc5i = dram.tile([128, 8], F32, tag="cc5i")
        cc5o = dram.tile([128, 8], F32, tag="cc5o")
        nc.gpsimd.dma_start(out=cc5i[:], in_=st5[:])
        nc.gpsimd.collective_compute(
            "AllReduce", ALU.add, replica_groups=[list(range(NCORES))],
            ins=[cc5i[:].opt()], outs=[cc5o[:].opt()])
        g5 = stat_pool.tile([128, 8], F32)
        nc.sync.dma_start(out=g5[:], in_=cc5o[:])
        mean5 = stat_pool.tile([128, 4], F32)
        nc.scalar.activation(out=mean5[:], in_=g5[:, 0:4], func=AF.Copy,
                             bias=0.0, scale=INV_N)
        ex25 = stat_pool.tile([128, 4], F32)
        nc.scalar.activation(out=ex25[:], in_=g5[:, 4:8], func=AF.Copy,
                             bias=0.0, scale=INV_N)
        var5 = stat_pool.tile([128, 4], F32)
        nc.vector.tensor_mul(out=var5[:], in0=mean5[:], in1=mean5[:])
        nc.vector.tensor_sub(out=var5[:], in0=ex25[:], in1=var5[:])
        std5 = stat_pool.tile([128, 4], F32)
        nc.scalar.activation(out=std5[:], in_=var5[:], func=AF.Sqrt,
                             bias=eps_sb[:, 0:1], scale=1.0)
        nc.vector.reciprocal(out=std5[:], in_=std5[:])
        aa5 = stat_pool.tile([128, 4], F32)
        nc.vector.tensor_mul(out=aa5[:], in0=std5[:], in1=gam_sb[:])
        dd5 = stat_pool.tile([128, 4], F32)
        nc.vector.tensor_mul(out=dd5[:], in0=mean5[:], in1=aa5[:])
        nc.vector.tensor_sub(out=dd5[:], in0=bet_sb[:], in1=dd5[:])

        a5_pool = popen("a5", 1, side="right")
        a5 = a5_pool.tile([128, 4, B, 10, 34], F32R)
        for g in range(4):
            for r in (0, 9):
                nc.gpsimd.memset(a5[:, g, :, r, :].bitcast(F32), 0.0)
            for c in (0, 33):
                nc.gpsimd.memset(a5[:, g, :, :, c].bitcast(F32), 0.0)
        for m in range(4):
            nc.scalar.activation(
                out=a5[:, m, :, 1:9, 1:33], in_=c5[:, m],
                func=AF.Identity, bias=dd5[:, m:m + 1], scale=aa5[:, m:m + 1])
        pclose(c5_pool)
        if debug:
            for m in range(4):
                for b in range(B):
                    nc.sync.dma_start(
                        out=_ap(dbg["a5"], m * 2048 + b * 256,
                                [[8192, 128], [32, 8], [1, 32]]),
                        in_=a5[:, m, b, 1:9, 1:33].bitcast(F32))

        # ---- conv6 -> c6; stats6; pool6 -> c6p; BN6 on pooled ----
        c6_pool = popen("c6", 1, side="left")
        c6 = c6_pool.tile([128, 4, B, 8, 32], F32)
        for wave in range(2):
            bps = [2 * wave, 2 * wave + 1]
            pts = {}
            for m in range(4):
                for bp in bps:
                    pts[(m, bp)] = psum.tile([128, 2, 8, 32], F32, tag="ps",
                                             name=f"ps6_{m}_{bp}")
            for g in range(4):
                i = wave * 4 + g
                w6g = w6tiles.pop((wave, g))
                if i + 2 < len(w6seq):
                    load_w6(i + 2)
                for tap in range(9):
                    kh, kw = tap // 3, tap % 3
                    for m in range(4):
                        for bp in bps:
                            b0 = 2 * bp
                            i = g * 9 + tap
                            nc.tensor.matmul(
                                pts[(m, bp)][:],
                                w6g[:, tap, 128 * m:128 * m + 128],
                                a5[:, g, b0:b0 + 2, kh:kh + 8, kw:kw + 32],
                                start=(i == 0), stop=(i == 35))
            for m in range(4):
                for bp in bps:
                    b0 = 2 * bp
                    nc.scalar.activation(
                        out=c6[:, m, b0:b0 + 2, :, :], in_=pts[(m, bp)][:],
                        func=AF.Identity, bias=b6_sb[:, m:m + 1], scale=1.0)
        pclose(a5_pool)

        # stats6 + pool6 in parallel
        st6 = stat_pool.tile([128, 8], F32)
        nc.vector.tensor_reduce(
            out=st6[:, 0:4], in_=c6[:].rearrange("p g b h w -> p g (b h w)"),
            axis=AX.X, op=ALU.add)
        for m in range(4):
            scr = scr_pool.tile([128, 2048], F32, tag="scr")
            nc.scalar.activation(
                out=scr[:], in_=c6[:, m].rearrange("p b h w -> p (b h w)"),
                func=AF.Square, bias=0.0, scale=1.0,
                accum_out=st6[:, 4 + m:5 + m])
        cc6i = dram.tile([128, 8], F32, tag="cc6i")
        cc6o = dram.tile([128, 8], F32, tag="cc6o")
        nc.gpsimd.dma_start(out=cc6i[:], in_=st6[:])
        nc.gpsimd.collective_compute(
            "AllReduce", ALU.add, replica_groups=[list(range(NCORES))],
            ins=[cc6i[:].opt()], outs=[cc6o[:].opt()])

        c6p_pool = popen("c6p", 1, side="right")
        wD = popen("wD", 1, side="right")
        w7_sb = wD.tile([128, 4, 4, 512], F32R)
        nc.gpsimd.dma_start(
            out=w7_sb[:],
            in_=_ap(w7T, 0, [[512, 128], [4 * 128 * 512, 4], [128 * 512, 4],
                             [1, 512]]).bitcast(F32R))
        c6p = c6p_pool.tile([128, 4, B, 4, 17], F32R)
        for g in range(4):
            nc.gpsimd.memset(c6p[:, g, :, :, 16].bitcast(F32), 0.0)
        for m in range(4):
            c6v = c6[:, m].rearrange("p b (h two) (w v) -> p b h two w v",
                                     two=2, v=2)
            nc.vector.tensor_max(out=c6p[:, m, :, :, 0:16], in0=c6v[:, :, :, 0, :, 0],
                                 in1=c6v[:, :, :, 0, :, 1])

        g6 = stat_pool.tile([128, 8], F32)
        nc.sync.dma_start(out=g6[:], in_=cc6o[:])
        mean6 = stat_pool.tile([128, 4], F32)
        nc.scalar.activation(out=mean6[:], in_=g6[:, 0:4], func=AF.Copy,
                             bias=0.0, scale=INV_N)
        ex26 = stat_pool.tile([128, 4], F32)
        nc.scalar.activation(out=ex26[:], in_=g6[:, 4:8], func=AF.Copy,
                             bias=0.0, scale=INV_N)
        var6 = stat_pool.tile([128, 4], F32)
        nc.vector.tensor_mul(out=var6[:], in0=mean6[:], in1=mean6[:])
        nc.vector.tensor_sub(out=var6[:], in0=ex26[:], in1=var6[:])
        std6 = stat_pool.tile([128, 4], F32)
        nc.scalar.activation(out=std6[:], in_=var6[:], func=AF.Sqrt,
                             bias=eps_sb[:, 0:1], scale=1.0)
        nc.vector.reciprocal(out=std6[:], in_=std6[:])
        aa6 = stat_pool.tile([128, 4], F32)
        nc.vector.tensor_mul(out=aa6[:], in0=std6[:], in1=gam_sb[:])
        dd6 = stat_pool.tile([128, 4], F32)
        nc.vector.tensor_mul(out=dd6[:], in0=mean6[:], in1=aa6[:])
        nc.vector.tensor_sub(out=dd6[:], in0=bet_sb[:], in1=dd6[:])
        for m in range(4):
            nc.scalar.activation(
                out=c6p[:, m], in_=c6p[:, m].bitcast(F32),
                func=AF.Identity, bias=dd6[:, m:m + 1], scale=aa6[:, m:m + 1])
        pclose(c6_pool)
        pclose(scr_pool)
        pclose(stat_pool)
        pclose(wC)
        if debug:
            for g in range(4):
                for b in range(B):
                    nc.sync.dma_start(
                        out=_ap(dbg["c6p"], g * 512 + b * 64,
                                [[2048, 128], [16, 4], [1, 16]]),
                        in_=c6p[:, g, b, :, 0:16].bitcast(F32))

        # ---- wE early: LSTM weights load overlaps conv7 ----
        wE = popen("wE", 1, side="left")
        wih_sb = wE.tile([128, 2, 8, 1024], F32)
        nc.gpsimd.dma_start(
            out=wih_sb[:],
            in_=_ap(wihT, 0, [[1024, 128], [8 * 128 * 1024, 2], [128 * 1024, 8],
                              [1, 1024]]))
        whh_sb = wE.tile([128, 2, 2, 1024], F16)
        nc.gpsimd.dma_start(
            out=whh_sb[:],
            in_=_ap(whhT, 0, [[1024, 128], [2 * 128 * 1024, 2], [128 * 1024, 2],
                              [1, 1024]]))
        lb_sb = wE.tile([128, 2, 8], F32)
        nc.sync.dma_start(out=lb_sb[:], in_=lbias)

        # ---- conv7 (VALID 2x2) -> c7 [128, 4, B, 3, 16] ----
        c7_pool = popen("c7", 1, side="left")
        c7 = c7_pool.tile([128, 4, B, 3, 16], F32)
        for m in range(4):
            pt = psum.tile([128, 8, 3, 16], F32, tag="ps")
            for g in range(4):
                for tap in range(4):
                    kh, kw = tap // 2, tap % 2
                    i = g * 4 + tap
                    nc.tensor.matmul(
                        pt[:], w7_sb[:, g, tap, 128 * m:128 * m + 128],
                        c6p[:, g, :, kh:kh + 3, kw:kw + 16],
                        start=(i == 0), stop=(i == 15))
            nc.scalar.activation(out=c7[:, m], in_=pt[:], func=AF.Identity,
                                 bias=b7_sb[:, m:m + 1], scale=1.0)
        pclose(wD)
        pclose(c6p_pool)
        if debug:
            nc.sync.dma_start(
                out=_ap(dbg["c7"], 0, [[1536, 128], [1, 1536]]),
                in_=c7[:].rearrange("p g b h w -> p (g b h w)"))

        # ---- LSTM ----
        ls = popen("ls", 1, side="right")
        xg = ls.tile([128, 2, 8, 8, 15], F32)
        hs = ls.tile([128, 2, 2, 8, 15], F16)
        hs32 = ls.tile([128, 2, 2, 8, 15], F32)
        cst = ls.tile([128, 2, 2, 8], F32)
        gp = popen("gp", 4, side="right")
        tp = popen("tp", 4, side="right")

        for dr in range(2):
            for m in range(8):
                pt = psum.tile([128, 8, 16], F32, tag="ps")
                for gd in range(8):
                    nc.tensor.matmul(
                        pt[:], wih_sb[:, dr, gd, 128 * m:128 * m + 128],
                        c7[:, gd % 4, :, gd // 4, :],
                        start=(gd == 0), stop=(gd == 7))
                nc.scalar.activation(
                    out=xg[:, dr, m],
                    in_=pt[:, :, 0:15],
                    func=AF.Identity, bias=lb_sb[:, dr, m:m + 1], scale=1.0)
        if debug:
            nc.sync.dma_start(
                out=_ap(dbg["xg"], 0, [[1920, 128], [1, 1920]]),
                in_=xg[:].rearrange("p d m b t -> p (d m b t)"))

        for t in range(15):
            for dr in range(2):
                tt = t if dr == 0 else 14 - t
                ga = gp.tile([128, 8, 8], F32, tag="ga")
                if t == 0:
                    nc.vector.tensor_copy(out=ga[:], in_=xg[:, dr, :, :, tt])
                else:
                    tprev = tt - 1 if dr == 0 else tt + 1
                    pr = psum.tile([128, 8, 8], F32, tag="ps")
                    for m in range(8):
                        for gh in range(2):
                            nc.tensor.matmul(
                                pr[:, m, :],
                                whh_sb[:, dr, gh, 128 * m:128 * m + 128],
                                hs[:, dr, gh, :, tprev],
                                start=(gh == 0), stop=(gh == 1),
                                skip_group_check=True)
                    nc.vector.tensor_add(out=ga[:], in0=pr[:],
                                         in1=xg[:, dr, :, :, tt])
                nc.scalar.activation(out=ga[:, 0:6, :], in_=ga[:, 0:6, :],
                                     func=AF.Sigmoid, bias=0.0, scale=1.0)
                nc.scalar.activation(out=ga[:, 6:8, :], in_=ga[:, 6:8, :],
                                     func=AF.Tanh, bias=0.0, scale=1.0)
                cs = cst[:, dr]
                if t == 0:
                    nc.vector.tensor_mul(out=cs, in0=ga[:, 0:2, :],
                                         in1=ga[:, 6:8, :])
                else:
                    t1 = tp.tile([128, 2, 8], F32, tag="t1")
                    nc.vector.tensor_mul(out=t1[:], in0=ga[:, 0:2, :],
                                         in1=ga[:, 6:8, :])
                    t2 = tp.tile([128, 2, 8], F32, tag="t2")
                    nc.vector.tensor_mul(out=t2[:], in0=ga[:, 2:4, :], in1=cs)
                    nc.vector.tensor_add(out=cs, in0=t1[:], in1=t2[:])
                th = tp.tile([128, 2, 8], F32, tag="th")
                nc.scalar.activation(out=th[:], in_=cs, func=AF.Tanh,
                                     bias=0.0, scale=1.0)
                nc.vector.tensor_mul(out=hs[:, dr, :, :, tt],
                                     in0=ga[:, 4:6, :], in1=th[:])
                nc.vector.tensor_mul(out=hs32[:, dr, :, :, tt],
                                     in0=ga[:, 4:6, :], in1=th[:])

        if debug:
            nc.sync.dma_start(
                out=_ap(dbg["hs"], 0, [[480, 128], [1, 480]]),
                in_=hs32[:].rearrange("p d g b t -> p (d g b t)"))

        # ---- output DMA: out[b, t, 256*dr + 128*gh + p] ----
        for dr in range(2):
            for gh in range(2):
                nc.sync.dma_start(
                    out=_ap(out, 256 * dr + 128 * gh,
                            [[1, 128], [512, 120]]),
                    in_=hs32[:, dr, gh].rearrange("p b t -> p (b t)"))

        for p in reversed(list(opened)):
            pclose(p)

    nc.compile()
    return nc


def prep_inputs(inputs, core):
    """Host-side: shard + transform weights for one core."""
    d = {}
    x = np.asarray(inputs["x"], dtype=np.float32)
    xs = x[core * B:(core + 1) * B, 0]          # (8, 64, 256)
    xp = np.zeros((B, 66, 258), np.float32)
    xp[:, 1:65, 1:257] = xs
    xim = np.empty((B, 9, 64, 256), np.float32)
    for kh in range(3):
        for kw in range(3):
            xim[:, kh * 3 + kw] = xp[:, kh:kh + 64, kw:kw + 256]
    d["xim"] = xim

    w1 = np.asarray(inputs["w1"], np.float32)   # (64,1,3,3)
    d["w1T"] = np.ascontiguousarray(
        w1[:, 0].reshape(64, 9).T)              # (9, 64)
    d["b1"] = np.asarray(inputs["b1"], np.float32).reshape(64, 1)

    w2 = np.asarray(inputs["w2"], np.float32)   # (128,64,3,3)
    w2p = np.zeros((3, 128, 128), np.float32)
    for kw in range(3):
        w2p[kw, 0:64] = w2[:, :, 0, kw].T
        w2p[kw, 64:128] = w2[:, :, 1, kw].T
    d["w2p"] = w2p
    d["w2s"] = np.ascontiguousarray(
        np.transpose(w2[:, :, 2, :], (2, 1, 0)))  # (3, 64, 128)
    d["b2"] = np.asarray(inputs["b2"], np.float32).reshape(1, 128).T.copy()

    def wT(w, gK, cout):
        # w (O, I, 3, 3) -> (gK, 9, 128, O)
        o, i_, kh, kw = w.shape
        r = np.transpose(w, (2, 3, 1, 0)).reshape(kh * kw, gK, 128, o)
        return np.ascontiguousarray(np.transpose(r, (1, 0, 2, 3)))

    d["w3T"] = wT(np.asarray(inputs["w3"], np.float32), 1, 256)
    d["w4T"] = wT(np.asarray(inputs["w4"], np.float32), 2, 256)
    d["w5T"] = wT(np.asarray(inputs["w5"], np.float32), 2, 512)
    d["w6T"] = wT(np.asarray(inputs["w6"], np.float32), 4, 512)
    w7 = np.asarray(inputs["w7"], np.float32)   # (512,512,2,2)
    r7 = np.transpose(w7, (2, 3, 1, 0)).reshape(4, 4, 128, 512)
    d["w7T"] = np.ascontiguousarray(np.transpose(r7, (1, 0, 2, 3)))
    for k, g in (("b3", 2), ("b4", 2), ("b5", 4), ("b6", 4), ("b7", 4)):
        src = "b" + k[1]
        d[k] = np.ascontiguousarray(
            np.asarray(inputs[src], np.float32).reshape(g, 128).T)
    d["gam"] = np.ascontiguousarray(
        np.asarray(inputs["gamma"], np.float32).reshape(4, 128).T)
    d["bet"] = np.ascontiguousarray(
        np.asarray(inputs["beta"], np.float32).reshape(4, 128).T)

    # LSTM: d-column permutation dmap maps compute-chunk col 128*j+p to
    # reference D index 2*(128*(j%4)+p) + j//4
    j = np.arange(8)[:, None]
    p = np.arange(128)[None, :]
    dmap = (2 * (128 * (j % 4) + p) + j // 4).reshape(-1)
    wih = np.stack([np.asarray(inputs["Wih_f"], np.float32),
                    np.asarray(inputs["Wih_b"], np.float32)])
    whh = np.stack([np.asarray(inputs["Whh_f"], np.float32),
                    np.asarray(inputs["Whh_b"], np.float32)])
    wihp = wih[:, PERM4H][:, :, dmap]           # (2, 1024, 1024)
    d["wihT"] = np.ascontiguousarray(
        np.transpose(wihp, (0, 2, 1)).reshape(2, 8, 128, 1024))
    whhp = whh[:, PERM4H]                       # (2, 1024, 256)
    d["whhT"] = np.ascontiguousarray(
        np.transpose(whhp, (0, 2, 1)).reshape(2, 2, 128, 1024)).astype(np.float16)
    lb = (np.stack([np.asarray(inputs["bih_f"], np.float32),
                    np.asarray(inputs["bih_b"], np.float32)])
          + np.stack([np.asarray(inputs["bhh_f"], np.float32),
                      np.asarray(inputs["bhh_b"], np.float32)]))
    lbp = lb[:, PERM4H].reshape(2, 8, 128)      # (dir, m, p)
    d["lbias"] = np.ascontiguousarray(np.transpose(lbp, (2, 0, 1)))
    return d


_NC_CACHE = {}


def kernel(**inputs):
    key = "debug" if inputs.pop("_debug", False) else "main"
    if key not in _NC_CACHE:
        _NC_CACHE[key] = build(debug=(key == "debug"))
    nc = _NC_CACHE[key]
    in_maps = [prep_inputs(inputs, c) for c in range(NCORES)]
    res = bass_utils.run_bass_kernel_spmd(nc, in_maps,
                                          core_ids=list(range(NCORES)))
    out = np.concatenate([res.results[c]["out"] for c in range(NCORES)], axis=0)
    kernel.last_results = res
    return out


# revision 31
# speedup vs baseline: 1.7561x; 1.3445x over previous
"""CaptchaCRNN Trainium2 kernel: 7 convs + 2 train-mode BN + maxpools + biLSTM.

Data-parallel over batch on 8 NeuronCores (8 images/core). BN batch stats are
globalized with a tiny AllReduce. Conv matmuls run in float32r (1 cyc/row).
"""
import sys

sys.path.insert(0, "/opt/trn_rl_repo")

import numpy as np
import concourse.bass as bass
import concourse.bacc as bacc
import concourse.tile as tile
from concourse import masks
from concourse import mybir
from concourse import bass_utils

F32 = mybir.dt.float32
F16 = mybir.dt.float16
F32R = mybir.dt.float32r
AF = mybir.ActivationFunctionType
ALU = mybir.AluOpType
AX = mybir.AxisListType

NCORES = 8
B = 8          # images per core
EPS = 1e-5
INV_N = 1.0 / (64 * 8 * 32)   # BN normalizer: full batch 64 x H8 x W32

# 4H gate permutation: torch order [i,f,g,o] -> compute order [i,f,o,g]
PERM4H = np.r_[0:512, 768:1024, 512:768]


def _ap(obj, offset, dims):
    base = obj if isinstance(obj, bass.AP) else obj[:]
    return bass.AP(tensor=base.tensor, offset=base.offset + offset,
                   ap=[list(d) for d in dims])


def build(debug=False):
    nc = bacc.Bacc("TRN2", target_bir_lowering=False, debug=False,
                   enable_asserts=True, num_devices=NCORES)

    def din(name, shape):
        return nc.dram_tensor(name, list(shape), F32, kind="ExternalInput").ap()

    def dout(name, shape):
        return nc.dram_tensor(name, list(shape), F32, kind="ExternalOutput").ap()

    xim = din("xim", (B, 9, 64, 256))
    w1T = din("w1T", (9, 64))
    b1 = din("b1", (64, 1))
    w2p = din("w2p", (3, 128, 128))
    w2s = din("w2s", (3, 64, 128))
    w3T = din("w3T", (1, 9, 128, 256))
    w4T = din("w4T", (2, 9, 128, 256))
    w5T = din("w5T", (2, 9, 128, 512))
    w6T = din("w6T", (4, 9, 128, 512))
    w7T = din("w7T", (4, 4, 128, 512))
    b2 = din("b2", (128, 1))
    b3 = din("b3", (128, 2))
    b4 = din("b4", (128, 2))
    b5 = din("b5", (128, 4))
    b6 = din("b6", (128, 4))
    b7 = din("b7", (128, 4))
    gam = din("gam", (128, 4))
    bet = din("bet", (128, 4))
    wihT = din("wihT", (2, 8, 128, 1024))
    whhT = nc.dram_tensor("whhT", [2, 2, 128, 1024], mybir.dt.float16,
                          kind="ExternalInput").ap()
    lbias = din("lbias", (128, 2, 8))
    out = dout("out", (B, 15, 512))

    dbg = {}
    if debug:
        dbg["a2"] = dout("dbg_a2", (128, 8, 16, 64))
        dbg["a4"] = dout("dbg_a4", (128, 2, 8, 8, 32))
        dbg["a5"] = dout("dbg_a5", (128, 4, 8, 8, 32))
        dbg["c6p"] = dout("dbg_c6p", (128, 4, 8, 4, 16))
        dbg["c7"] = dout("dbg_c7", (128, 4, 8, 3, 16))
        dbg["xg"] = dout("dbg_xg", (128, 2, 8, 8, 15))
        dbg["hs"] = dout("dbg_hs", (128, 2, 2, 8, 15))

    with tile.TileContext(nc) as tc:
        opened = []

        def popen(name, bufs, space="SBUF", side=None):
            cm = tc.tile_pool(name=name, bufs=bufs, space=space, side=side)
            p = cm.__enter__()
            p._cm = cm
            opened.append(p)
            return p

        def pclose(p):
            p._cm.__exit__(None, None, None)
            opened.remove(p)

        const = popen("const", 1, side="left")
        psum = popen("psum", 8, space="PSUM")
        dram = popen("dram", 1, space="DRAM")

        # ---- constants ----
        b1_sb = const.tile([64, 1], F32)
        nc.sync.dma_start(out=b1_sb[:], in_=b1)
        b2_sb = const.tile([128, 1], F32)
        nc.sync.dma_start(out=b2_sb[:], in_=b2)
        b3_sb = const.tile([128, 2], F32)
        nc.sync.dma_start(out=b3_sb[:], in_=b3)
        b4_sb = const.tile([128, 2], F32)
        nc.sync.dma_start(out=b4_sb[:], in_=b4)
        b5_sb = const.tile([128, 4], F32)
        nc.sync.dma_start(out=b5_sb[:], in_=b5)
        b6_sb = const.tile([128, 4], F32)
        nc.sync.dma_start(out=b6_sb[:], in_=b6)
        b7_sb = const.tile([128, 4], F32)
        nc.sync.dma_start(out=b7_sb[:], in_=b7)
        gam_sb = const.tile([128, 4], F32)
        nc.sync.dma_start(out=gam_sb[:], in_=gam)
        bet_sb = const.tile([128, 4], F32)
        nc.sync.dma_start(out=bet_sb[:], in_=bet)
        eps_sb = const.tile([128, 1], F32)
        nc.vector.memset(eps_sb[:], EPS)
        ident = const.tile([128, 128], F32)
        masks.make_identity(nc, ident[:])

        # ---- conv1 weights ----
        wA = popen("wA", 1, side="left")
        w1_sb = wA.tile([9, 64], F32R)
        nc.sync.dma_start(out=w1_sb[:], in_=w1T.bitcast(F32R))
        w2p_sb = wA.tile([128, 3, 128], F32R)
        # w2p dram (3,128,128): partition = axis1
        nc.sync.dma_start(
            out=w2p_sb[:],
            in_=_ap(w2p, 0, [[128, 128], [128 * 128, 3], [1, 128]]).bitcast(F32R))
        w2s_sb = wA.tile([128, 3, 128], F32R)
        nc.sync.dma_start(
            out=w2s_sb[64:128, :, :],
            in_=_ap(w2s, 0, [[128, 64], [64 * 128, 3], [1, 128]]).bitcast(F32R))
        w3_sb = wA.tile([128, 9, 256], F32R)
        nc.sync.dma_start(
            out=w3_sb[:],
            in_=_ap(w3T, 0, [[256, 128], [128 * 256, 9], [1, 256]]).bitcast(F32R))
        w4_sb = wA.tile([128, 2, 9, 256], F32R)
        nc.sync.dma_start(
            out=w4_sb[:],
            in_=_ap(w4T, 0, [[256, 128], [9 * 128 * 256, 2], [128 * 256, 9],
                             [1, 256]]).bitcast(F32R))

        # ---- conv1 + pool1 -> p1d DRAM (B, 64, 32, 128) ----
        p1d = dram.tile([B, 64, 32, 128], F32)
        rhs1 = popen("rhs1", 2, side="left")
        st1 = popen("st1", 2, side="left")
        for b in range(B):
            for q in range(4):
                r0 = 16 * q
                rt = rhs1.tile([9, 16, 256], F32R, tag="rhs1")
                nc.sync.dma_start(
                    out=rt[:],
                    in_=_ap(xim, b * (9 * 64 * 256) + r0 * 256,
                            [[64 * 256, 9], [1, 16 * 256]]).bitcast(F32R))
                s1q = st1.tile([64, 16, 256], F32, tag="s1q")
                rv = rt[:].rearrange("k (j t) w -> k j (t w)", t=2)
                for j in range(8):
                    pt = psum.tile([64, 512], F32, tag="ps")
                    nc.tensor.matmul(pt[:], w1_sb[:], rv[:, j, :],
                                     start=True, stop=True)
                    nc.scalar.activation(
                        out=s1q[:, 2 * j:2 * j + 2, :].rearrange("k a b -> k (a b)"),
                        in_=pt[:], func=AF.Identity, bias=b1_sb[:, 0:1], scale=1.0)
                wq = st1.tile([64, 16, 128], F32, tag="wq")
                s1v = s1q[:].rearrange("k h (w two) -> k h w two", two=2)
                nc.vector.tensor_max(out=wq[:], in0=s1v[:, :, :, 0],
                                     in1=s1v[:, :, :, 1])
                hq = st1.tile([64, 8, 128], F32, tag="hq")
                wv = wq[:].rearrange("k (h two) w -> k h two w", two=2)
                nc.vector.tensor_max(out=hq[:], in0=wv[:, :, 0, :],
                                     in1=wv[:, :, 1, :])
                nc.sync.dma_start(
                    out=_ap(p1d, b * (64 * 32 * 128) + (8 * q) * 128,
                            [[32 * 128, 64], [128, 8], [1, 128]]),
                    in_=hq[:])
        pclose(st1)
        pclose(rhs1)

        # ---- conv2 + pool2 -> a2p [128, B, 18, 66] ----
        a2p_pool = popen("a2p", 1, side="right")
        a2p = a2p_pool.tile([128, B, 18, 66], F32R)
        # zero borders of a2p once
        for r in (0, 17):
            nc.gpsimd.memset(a2p[:, :, r, :].bitcast(F32), 0.0)
        for c in (0, 65):
            nc.gpsimd.memset(a2p[:, :, :, c].bitcast(F32), 0.0)

        a1b_pool = popen("a1b", 2, side="left")
        st2 = popen("st2", 3, side="left")
        for b in range(B):
            a1b = a1b_pool.tile([128, 34, 130], F32R, tag="a1b")
            for r in (0, 33):
                nc.gpsimd.memset(a1b[0:64, r, :].bitcast(F32), 0.0)
            nc.gpsimd.memset(a1b[64:128, 32:34, :].bitcast(F32), 0.0)
            for c in (0, 129):
                nc.gpsimd.memset(a1b[:, :, c].bitcast(F32), 0.0)
            src = _ap(p1d, b * (64 * 32 * 128),
                      [[32 * 128, 64], [128, 32], [1, 128]]).bitcast(F32R)
            nc.sync.dma_start(out=a1b[0:64, 1:33, 1:129], in_=src)
            nc.sync.dma_start(out=a1b[64:128, 0:32, 1:129], in_=src)
            for n in range(8):
                h0 = 4 * n
                pt = psum.tile([128, 4, 128], F32, tag="ps")
                for kw in range(3):
                    nc.tensor.matmul(
                        pt[:], w2p_sb[:, kw, :],
                        a1b[0:128, h0:h0 + 4, kw:kw + 128],
                        start=(kw == 0), stop=False)
                for kw in range(3):
                    nc.tensor.matmul(
                        pt[:], w2s_sb[64:128, kw, :],
                        a1b[64:128, h0 + 1:h0 + 5, kw:kw + 128],
                        start=False, stop=(kw == 2))
                s2 = st2.tile([128, 4, 128], F32, tag="s2")
                nc.scalar.copy(out=s2[:].rearrange("p a b -> p (a b)"), in_=pt[:])
                w2m = st2.tile([128, 4, 64], F32, tag="w2m")
                s2v = s2[:].rearrange("p h (w two) -> p h w two", two=2)
                nc.vector.tensor_max(out=w2m[:], in0=s2v[:, :, :, 0],
                                     in1=s2v[:, :, :, 1])
                wv = w2m[:].rearrange("p (h two) w -> p h two w", two=2)
                nc.vector.tensor_max(out=a2p[:, b, 1 + 2 * n:3 + 2 * n, 1:65],
                                     in0=wv[:, :, 0, :], in1=wv[:, :, 1, :])
        # bias b2 on interior
        nc.scalar.activation(out=a2p[:, :, 1:17, 1:65], in_=a2p[:, :, 1:17, 1:65].bitcast(F32),
                             func=AF.Identity, bias=b2_sb[:, 0:1], scale=1.0)
        pclose(st2)
        pclose(a1b_pool)
        if debug:
            for b in range(B):
                nc.sync.dma_start(
                    out=_ap(dbg["a2"], b * 1024, [[8192, 128], [64, 16], [1, 64]]),
                    in_=a2p[:, b, 1:17, 1:65].bitcast(F32))

        # ---- conv3 -> a3 [128, 2, B, 18, 66] ----
        a3_pool = popen("a3", 1, side="left")
        a3 = a3_pool.tile([128, 2, B, 18, 66], F32R)
        for g in range(2):
            for r in (0, 17):
                nc.gpsimd.memset(a3[:, g, :, r, :].bitcast(F32), 0.0)
            for c in (0, 65):
                nc.gpsimd.memset(a3[:, g, :, :, c].bitcast(F32), 0.0)
        for b in range(B):
            for m in range(2):
                for n in range(2):
                    pt = psum.tile([128, 8, 64], F32, tag="ps")
                    for kh in range(3):
                        for kw in range(3):
                            tap = kh * 3 + kw
                            nc.tensor.matmul(
                                pt[:], w3_sb[:, tap, 128 * m:128 * m + 128],
                                a2p[:, b, 8 * n + kh:8 * n + kh + 8, kw:kw + 64],
                                start=(tap == 0), stop=(tap == 8))
                    nc.scalar.activation(
                        out=a3[:, m, b, 1 + 8 * n:9 + 8 * n, 1:65],
                        in_=pt[:], func=AF.Identity, bias=b3_sb[:, m:m + 1],
                        scale=1.0)
        pclose(a2p_pool)

        # ---- conv4 + pool4 -> a4p [128, 2, B, 10, 34] ----
        a4p_pool = popen("a4p", 1, side="right")
        a4p = a4p_pool.tile([128, 2, B, 10, 34], F32R)
        for g in range(2):
            for r in (0, 9):
                nc.gpsimd.memset(a4p[:, g, :, r, :].bitcast(F32), 0.0)
            for c in (0, 33):
                nc.gpsimd.memset(a4p[:, g, :, :, c].bitcast(F32), 0.0)
        # prefetch w5
        wB = popen("wB", 1, side="right")
        w5_sb = wB.tile([128, 2, 9, 512], F32R)
        nc.gpsimd.dma_start(
            out=w5_sb[:],
            in_=_ap(w5T, 0, [[512, 128], [9 * 128 * 512, 2], [128 * 512, 9],
                             [1, 512]]).bitcast(F32R))
        st4 = popen("st4", 3, side="right")
        for b in range(B):
            for m in range(2):
                for n in range(2):
                    pt = psum.tile([128, 8, 64], F32, tag="ps")
                    for g in range(2):
                        for kh in range(3):
                            for kw in range(3):
                                i = g * 9 + kh * 3 + kw
                                nc.tensor.matmul(
                                    pt[:], w4_sb[:, g, kh * 3 + kw,
                                                 128 * m:128 * m + 128],
                                    a3[:, g, b, 8 * n + kh:8 * n + kh + 8,
                                       kw:kw + 64],
                                    start=(i == 0), stop=(i == 17))
                    s4 = st4.tile([128, 8, 64], F32, tag="s4")
                    nc.scalar.copy(out=s4[:].rearrange("p a b -> p (a b)"),
                                   in_=pt[:])
                    s4v = s4[:].rearrange("p (h two) (w v) -> p h two w v",
                                          two=2, v=2)
                    nc.vector.tensor_max(
                        out=a4p[:, m, b, 1 + 4 * n:5 + 4 * n, 1:33],
                        in0=s4v[:, :, 0, :, 0], in1=s4v[:, :, 0, :, 1])
        for m in range(2):
            nc.scalar.activation(out=a4p[:, m, :, 1:9, 1:33],
                                 in_=a4p[:, m, :, 1:9, 1:33].bitcast(F32),
                                 func=AF.Identity, bias=b4_sb[:, m:m + 1],
                                 scale=1.0)
        pclose(st4)
        pclose(a3_pool)
        pclose(wA)
        if debug:
            for m in range(2):
                for b in range(B):
                    nc.sync.dma_start(
                        out=_ap(dbg["a4"], m * 2048 + b * 256,
                                [[4096, 128], [32, 8], [1, 32]]),
                        in_=a4p[:, m, b, 1:9, 1:33].bitcast(F32))

        # ---- conv5 -> c5 [128, 4, B, 8, 32]; BN5 -> a5 [128, 4, B, 10, 34] ----
        wC = popen("wC", 2, side="left")
        w6seq = [(wv, g) for wv in range(2) for g in range(4)]
        w6tiles = {}

        def load_w6(i):
            wv, g = w6seq[i]
            t = wC.tile([128, 9, 512], F32R, tag="w6g", name=f"w6g_{wv}_{g}")
            nc.gpsimd.dma_start(
                out=t[:],
                in_=_ap(w6T, g * (9 * 128 * 512),
                        [[512, 128], [128 * 512, 9], [1, 512]]).bitcast(F32R))
            w6tiles[(wv, g)] = t

        load_w6(0)
        load_w6(1)

        stat_pool = popen("stat", 1, side="left")
        scr_pool = popen("scr", 2, side="left")
        c5_pool = popen("c5", 1, side="left")
        c5 = c5_pool.tile([128, 4, B, 8, 32], F32)
        for bp in range(4):
            b0 = 2 * bp
            for m in range(4):
                pt = psum.tile([128, 2, 8, 32], F32, tag="ps")
                for g in range(2):
                    for kh in range(3):
                        for kw in range(3):
                            i = g * 9 + kh * 3 + kw
                            nc.tensor.matmul(
                                pt[:], w5_sb[:, g, kh * 3 + kw,
                                             128 * m:128 * m + 128],
                                a4p[:, g, b0:b0 + 2, kh:kh + 8, kw:kw + 32],
                                start=(i == 0), stop=(i == 17))
                nc.scalar.activation(
                    out=c5[:, m, b0:b0 + 2, :, :], in_=pt[:],
                    func=AF.Identity, bias=b5_sb[:, m:m + 1], scale=1.0)
        pclose(wB)
        pclose(a4p_pool)

        # BN5 stats
        st5 = stat_pool.tile([128, 8], F32)
        nc.vector.tensor_reduce(
            out=st5[:, 0:4], in_=c5[:].rearrange("p g b h w -> p g (b h w)"),
            axis=AX.X, op=ALU.add)
        for m in range(4):
            scr = scr_pool.tile([128, 2048], F32, tag="scr")
            nc.scalar.activation(
                out=scr[:], in_=c5[:, m].rearrange("p b h w -> p (b h w)"),
                func=AF.Square, bias=0.0, scale=1.0,
                accum_out=st5[:, 4 + m:5 + m])
        cc5i = dram.tile([128, 8], F32, tag="cc5i")
        cc5o = dram.tile([128, 8], F32, tag="cc5o")
        nc.gpsimd.dma_start(out=cc5i[:], in_=st5[:])
        nc.gpsimd.collective_compute(
            "AllReduce", ALU.add, replica_groups=[list(range(NCORES))],
            ins=[cc5i[:].opt()], outs=[cc5o[:].opt()])
        g5 = stat_pool.tile([128, 8], F32)
        nc.sync.dma_start(out=g5[:], in_=cc5o[:])
        mean5 = stat_pool.tile([128, 4], F32)
        nc.scalar.activation(out=mean5[:], in_=g5[:, 0:4], func=AF.Copy,
                             bias=0.0, scale=INV_N)
        ex25 = stat_pool.tile([128, 4], F32)
        nc.scalar.activation(out=ex25[:], in_=g5[:, 4:8], func=AF.Copy,
                             bias=0.0, scale=INV_N)
        var5 = stat_pool.tile([128, 4], F32)
        nc.vector.tensor_mul(out=var5[:], in0=mean5[:], in1=mean5[:])
        nc.vector.tensor_sub(out=var5[:], in0=ex25[:], in1=var5[:])
        std5 = stat_pool.tile([128, 4], F32)
        nc.scalar.activation(out=std5[:], in_=var5[:], func=AF.Sqrt,
                             bias=eps_sb[:, 0:1], scale=1.0)
        nc.vector.reciprocal(out=std5[:], in_=std5[:])
        aa5 = stat_pool.tile([128, 4], F32)
        nc.vector.tensor_mul(out=aa5[:], in0=std5[:], in1=gam_sb[:])
        dd5 = stat_pool.tile([128, 4], F32)
        nc.vector.tensor_mul(out=dd5[:], in0=mean5[:], in1=aa5[:])
        nc.vector.tensor_sub(out=dd5[:], in0=bet_sb[:], in1=dd5[:])

        a5_pool = popen("a5", 1, side="right")
        a5 = a5_pool.tile([128, 4, B, 10, 34], F32R)
        for g in range(4):
            for r in (0, 9):
                nc.gpsimd.memset(a5[:, g, :, r, :].bitcast(F32), 0.0)
            for c in (0, 33):
                nc.gpsimd.memset(a5[:, g, :, :, c].bitcast(F32), 0.0)
        for m in range(4):
            nc.scalar.activation(
                out=a5[:, m, :, 1:9, 1:33], in_=c5[:, m],
                func=AF.Identity, bias=dd5[:, m:m + 1], scale=aa5[:, m:m + 1])
        pclose(c5_pool)
        if debug:
            for m in range(4):
                for b in range(B):
                    nc.sync.dma_start(
                        out=_ap(dbg["a5"], m * 2048 + b * 256,
                                [[8192, 128], [32, 8], [1, 32]]),
                        in_=a5[:, m, b, 1:9, 1:33].bitcast(F32))

        # ---- conv6 -> c6; stats6; pool6 -> c6p; BN6 on pooled ----
        c6_pool = popen("c6", 1, side="left")
        c6 = c6_pool.tile([128, 4, B, 8, 32], F32)
        for wave in range(2):
            bps = [2 * wave, 2 * wave + 1]
            pts = {}
            for m in range(4):
                for bp in bps:
                    pts[(m, bp)] = psum.tile([128, 2, 8, 32], F32, tag="ps",
                                             name=f"ps6_{m}_{bp}")
            for g in range(4):
                i = wave * 4 + g
                w6g = w6tiles.pop((wave, g))
                if i + 2 < len(w6seq):
                    load_w6(i + 2)
                for tap in range(9):
                    kh, kw = tap // 3, tap % 3
                    for m in range(4):
                        for bp in bps:
                            b0 = 2 * bp
                            i = g * 9 + tap
                            nc.tensor.matmul(
                                pts[(m, bp)][:],
                                w6g[:, tap, 128 * m:128 * m + 128],
                                a5[:, g, b0:b0 + 2, kh:kh + 8, kw:kw + 32],
                                start=(i == 0), stop=(i == 35))
            for m in range(4):
                for bp in bps:
                    b0 = 2 * bp
                    nc.scalar.activation(
                        out=c6[:, m, b0:b0 + 2, :, :], in_=pts[(m, bp)][:],
                        func=AF.Identity, bias=b6_sb[:, m:m + 1], scale=1.0)
        pclose(a5_pool)

        # stats6 + pool6 in parallel
        st6 = stat_pool.tile([128, 8], F32)
        nc.vector.tensor_reduce(
            out=st6[:, 0:4], in_=c6[:].rearrange("p g b h w -> p g (b h w)"),
            axis=AX.X, op=ALU.add)
        for m in range(4):
            scr = scr_pool.tile([128, 2048], F32, tag="scr")
            nc.scalar.activation(
                out=scr[:], in_=c6[:, m].rearrange("p b h w -> p (b h w)"),
                func=AF.Square, bias=0.0, scale=1.0,
                accum_out=st6[:, 4 + m:5 + m])
        cc6i = dram.tile([128, 8], F32, tag="cc6i")
        cc6o = dram.tile([128, 8], F32, tag="cc6o")
        nc.gpsimd.dma_start(out=cc6i[:], in_=st6[:])
        nc.gpsimd.collective_compute(
            "AllReduce", ALU.add, replica_groups=[list(range(NCORES))],
            ins=[cc6i[:].opt()], outs=[cc6o[:].opt()])

        c6p_pool = popen("c6p", 1, side="right")
        wD = popen("wD", 1, side="right")
        w7_sb = wD.tile([128, 4, 4, 512], F32R)
        nc.gpsimd.dma_start(
            out=w7_sb[:],
            in_=_ap(w7T, 0, [[512, 128], [4 * 128 * 512, 4], [128 * 512, 4],
                             [1, 512]]).bitcast(F32R))
        c6p = c6p_pool.tile([128, 4, B, 4, 17], F32R)
        for g in range(4):
            nc.gpsimd.memset(c6p[:, g, :, :, 16].bitcast(F32), 0.0)
        for m in range(4):
            c6v = c6[:, m].rearrange("p b (h two) (w v) -> p b h two w v",
                                     two=2, v=2)
            nc.vector.tensor_max(out=c6p[:, m, :, :, 0:16], in0=c6v[:, :, :, 0, :, 0],
                                 in1=c6v[:, :, :, 0, :, 1])

        g6 = stat_pool.tile([128, 8], F32)
        nc.sync.dma_start(out=g6[:], in_=cc6o[:])
        mean6 = stat_pool.tile([128, 4], F32)
        nc.scalar.activation(out=mean6[:], in_=g6[:, 0:4], func=AF.Copy,
                             bias=0.0, scale=INV_N)
        ex26 = stat_pool.tile([128, 4], F32)
        nc.scalar.activation(out=ex26[:], in_=g6[:, 4:8], func=AF.Copy,
                             bias=0.0, scale=INV_N)
        var6 = stat_pool.tile([128, 4], F32)
        nc.vector.tensor_mul(out=var6[:], in0=mean6[:], in1=mean6[:])
        nc.vector.tensor_sub(out=var6[:], in0=ex26[:], in1=var6[:])
        std6 = stat_pool.tile([128, 4], F32)
        nc.scalar.activation(out=std6[:], in_=var6[:], func=AF.Sqrt,
                             bias=eps_sb[:, 0:1], scale=1.0)
        nc.vector.reciprocal(out=std6[:], in_=std6[:])
        aa6 = stat_pool.tile([128, 4], F32)
        nc.vector.tensor_mul(out=aa6[:], in0=std6[:], in1=gam_sb[:])
        dd6 = stat_pool.tile([128, 4], F32)
        nc.vector.tensor_mul(out=dd6[:], in0=mean6[:], in1=aa6[:])
        nc.vector.tensor_sub(out=dd6[:], in0=bet_sb[:], in1=dd6[:])
        for m in range(4):
            nc.scalar.activation(
                out=c6p[:, m], in_=c6p[:, m].bitcast(F32),
                func=AF.Identity, bias=dd6[:, m:m + 1], scale=aa6[:, m:m + 1])
        pclose(c6_pool)
        pclose(scr_pool)
        pclose(stat_pool)
        pclose(wC)
        if debug:
            for g in range(4):
                for b in range(B):
                    nc.sync.dma_start(
                        out=_ap(dbg["c6p"], g * 512 + b * 64,
                                [[2048, 128], [16, 4], [1, 16]]),
                        in_=c6p[:, g, b, :, 0:16].bitcast(F32))

        # ---- wE early: LSTM weights load overlaps conv7 ----
        wE = popen("wE", 1, side="left")
        wih_sb = wE.tile([128, 2, 8, 1024], F32)
        nc.gpsimd.dma_start(
            out=wih_sb[:],
            in_=_ap(wihT, 0, [[1024, 128], [8 * 128 * 1024, 2], [128 * 1024, 8],
                              [1, 1024]]))
        whh_sb = wE.tile([128, 2, 2, 1024], F16)
        nc.gpsimd.dma_start(
            out=whh_sb[:],
            in_=_ap(whhT, 0, [[1024, 128], [2 * 128 * 1024, 2], [128 * 1024, 2],
                              [1, 1024]]))
        lb_sb = wE.tile([128, 2, 8], F32)
        nc.sync.dma_start(out=lb_sb[:], in_=lbias)

        # ---- conv7 (VALID 2x2) -> c7 [128, 4, B, 3, 16] ----
        c7_pool = popen("c7", 1, side="left")
        c7 = c7_pool.tile([128, 4, B, 3, 16], F32)
        for m in range(4):
            pt = psum.tile([128, 8, 3, 16], F32, tag="ps")
            for g in range(4):
                for tap in range(4):
                    kh, kw = tap // 2, tap % 2
                    i = g * 4 + tap
                    nc.tensor.matmul(
                        pt[:], w7_sb[:, g, tap, 128 * m:128 * m + 128],
                        c6p[:, g, :, kh:kh + 3, kw:kw + 16],
                        start=(i == 0), stop=(i == 15))
            nc.scalar.activation(out=c7[:, m], in_=pt[:], func=AF.Identity,
                                 bias=b7_sb[:, m:m + 1], scale=1.0)
        pclose(wD)
        pclose(c6p_pool)
        if debug:
            nc.sync.dma_start(
                out=_ap(dbg["c7"], 0, [[1536, 128], [1, 1536]]),
                in_=c7[:].rearrange("p g b h w -> p (g b h w)"))

        # ---- LSTM ----
        ls = popen("ls", 1, side="right")
        xg = ls.tile([128, 2, 8, 8, 15], F32)
        hs = ls.tile([128, 2, 2, 8, 15], F16)
        hs32 = ls.tile([128, 2, 2, 8, 15], F32)
        cst = ls.tile([128, 2, 2, 8], F32)
        gp = popen("gp", 4, side="right")
        tp = popen("tp", 4, side="right")

        for dr in range(2):
            for m in range(8):
                pt = psum.tile([128, 8, 16], F32, tag="ps")
                for gd in range(8):
                    nc.tensor.matmul(
                        pt[:], wih_sb[:, dr, gd, 128 * m:128 * m + 128],
                        c7[:, gd % 4, :, gd // 4, :],
                        start=(gd == 0), stop=(gd == 7))
                nc.scalar.activation(
                    out=xg[:, dr, m],
                    in_=pt[:, :, 0:15],
                    func=AF.Identity, bias=lb_sb[:, dr, m:m + 1], scale=1.0)
        if debug:
            nc.sync.dma_start(
                out=_ap(dbg["xg"], 0, [[1920, 128], [1, 1920]]),
                in_=xg[:].rearrange("p d m b t -> p (d m b t)"))

        for t in range(15):
            for dr in range(2):
                tt = t if dr == 0 else 14 - t
                ga = gp.tile([128, 8, 8], F32, tag="ga")
                if t == 0:
                    nc.vector.tensor_copy(out=ga[:], in_=xg[:, dr, :, :, tt])
                else:
                    tprev = tt - 1 if dr == 0 else tt + 1
                    pr = psum.tile([128, 8, 8], F32, tag="ps")
                    for m in range(8):
                        for gh in range(2):
                            nc.tensor.matmul(
                                pr[:, m, :],
                                whh_sb[:, dr, gh, 128 * m:128 * m + 128],
                                hs[:, dr, gh, :, tprev],
                                start=(gh == 0), stop=(gh == 1),
                                skip_group_check=True)
                    nc.vector.tensor_add(out=ga[:], in0=pr[:],
                                         in1=xg[:, dr, :, :, tt])
                nc.scalar.activation(out=ga[:, 0:6, :], in_=ga[:, 0:6, :],
                                     func=AF.Sigmoid, bias=0.0, scale=1.0)
                nc.scalar.activation(out=ga[:, 6:8, :], in_=ga[:, 6:8, :],
                                     func=AF.Tanh, bias=0.0, scale=1.0)
                cs = cst[:, dr]
                if t == 0:
                    nc.vector.tensor_mul(out=cs, in0=ga[:, 0:2, :],
                                         in1=ga[:, 6:8, :])
                else:
                    t1 = tp.tile([128, 2, 8], F32, tag="t1")
                    nc.vector.tensor_mul(out=t1[:], in0=ga[:, 0:2, :],
                                         in1=ga[:, 6:8, :])
                    t2 = tp.tile([128, 2, 8], F32, tag="t2")
                    nc.vector.tensor_mul(out=t2[:], in0=ga[:, 2:4, :], in1=cs)
                    nc.vector.tensor_add(out=cs, in0=t1[:], in1=t2[:])
                th = tp.tile([128, 2, 8], F32, tag="th")
                nc.scalar.activation(out=th[:], in_=cs, func=AF.Tanh,
                                     bias=0.0, scale=1.0)
                nc.vector.tensor_mul(out=hs[:, dr, :, :, tt],
                                     in0=ga[:, 4:6, :], in1=th[:])
                nc.vector.tensor_mul(out=hs32[:, dr, :, :, tt],
                                     in0=ga[:, 4:6, :], in1=th[:])

        if debug:
            nc.sync.dma_start(
                out=_ap(dbg["hs"], 0, [[480, 128], [1, 480]]),
                in_=hs32[:].rearrange("p d g b t -> p (d g b t)"))

        # ---- output: PE-transpose h to (b*t)-partitions, then contiguous DMA ----
        ptt = psum.tile([120, 512], F32, tag="ps")
        for dr in range(2):
            for gh in range(2):
                nc.tensor.transpose(
                    ptt[:, 128 * (2 * dr + gh):128 * (2 * dr + gh) + 128],
                    hs32[:, dr, gh].rearrange("p b t -> p (b t)"),
                    ident[:])
        outsb = ls.tile([120, 512], F32)
        nc.scalar.copy(out=outsb[:], in_=ptt[:])
        nc.sync.dma_start(out=_ap(out, 0, [[512, 120], [1, 512]]), in_=outsb[:])

        for p in reversed(list(opened)):
            pclose(p)

    nc.compile()
    return nc


def prep_inputs(inputs, core):
    """Host-side: shard + transform weights for one core."""
    d = {}
    x = np.asarray(inputs["x"], dtype=np.float32)
    xs = x[core * B:(core + 1) * B, 0]          # (8, 64, 256)
    xp = np.zeros((B, 66, 258), np.float32)
    xp[:, 1:65, 1:257] = xs
    xim = np.empty((B, 9, 64, 256), np.float32)
    for kh in range(3):
        for kw in range(3):
            xim[:, kh * 3 + kw] = xp[:, kh:kh + 64, kw:kw + 256]
    d["xim"] = xim

    w1 = np.asarray(inputs["w1"], np.float32)   # (64,1,3,3)
    d["w1T"] = np.ascontiguousarray(
        w1[:, 0].reshape(64, 9).T)              # (9, 64)
    d["b1"] = np.asarray(inputs["b1"], np.float32).reshape(64, 1)

    w2 = np.asarray(inputs["w2"], np.float32)   # (128,64,3,3)
    w2p = np.zeros((3, 128, 128), np.float32)
    for kw in range(3):
        w2p[kw, 0:64] = w2[:, :, 0, kw].T
        w2p[kw, 64:128] = w2[:, :, 1, kw].T
    d["w2p"] = w2p
    d["w2s"] = np.ascontiguousarray(
        np.transpose(w2[:, :, 2, :], (2, 1, 0)))  # (3, 64, 128)
    d["b2"] = np.asarray(inputs["b2"], np.float32).reshape(1, 128).T.copy()

    def wT(w, gK, cout):
        # w (O, I, 3, 3) -> (gK, 9, 128, O)
        o, i_, kh, kw = w.shape
        r = np.transpose(w, (2, 3, 1, 0)).reshape(kh * kw, gK, 128, o)
        return np.ascontiguousarray(np.transpose(r, (1, 0, 2, 3)))

    d["w3T"] = wT(np.asarray(inputs["w3"], np.float32), 1, 256)
    d["w4T"] = wT(np.asarray(inputs["w4"], np.float32), 2, 256)
    d["w5T"] = wT(np.asarray(inputs["w5"], np.float32), 2, 512)
    d["w6T"] = wT(np.asarray(inputs["w6"], np.float32), 4, 512)
    w7 = np.asarray(inputs["w7"], np.float32)   # (512,512,2,2)
    r7 = np.transpose(w7, (2, 3, 1, 0)).reshape(4, 4, 128, 512)
    d["w7T"] = np.ascontiguousarray(np.transpose(r7, (1, 0, 2, 3)))
    for k, g in (("b3", 2), ("b4", 2), ("b5", 4), ("b6", 4), ("b7", 4)):
        src = "b" + k[1]
        d[k] = np.ascontiguousarray(
            np.asarray(inputs[src], np.float32).reshape(g, 128).T)
    d["gam"] = np.ascontiguousarray(
        np.asarray(inputs["gamma"], np.float32).reshape(4, 128).T)
    d["bet"] = np.ascontiguousarray(
        np.asarray(inputs["beta"], np.float32).reshape(4, 128).T)

    # LSTM: d-column permutation dmap maps compute-chunk col 128*j+p to
    # reference D index 2*(128*(j%4)+p) + j//4
    j = np.arange(8)[:, None]
    p = np.arange(128)[None, :]
    dmap = (2 * (128 * (j % 4) + p) + j // 4).reshape(-1)
    wih = np.stack([np.asarray(inputs["Wih_f"], np.float32),
                    np.asarray(inputs["Wih_b"], np.float32)])
    whh = np.stack([np.asarray(inputs["Whh_f"], np.float32),
                    np.asarray(inputs["Whh_b"], np.float32)])
    wihp = wih[:, PERM4H][:, :, dmap]           # (2, 1024, 1024)
    d["wihT"] = np.ascontiguousarray(
        np.transpose(wihp, (0, 2, 1)).reshape(2, 8, 128, 1024))
    whhp = whh[:, PERM4H]                       # (2, 1024, 256)
    d["whhT"] = np.ascontiguousarray(
        np.transpose(whhp, (0, 2, 1)).reshape(2, 2, 128, 1024)).astype(np.float16)
    lb = (np.stack([np.asarray(inputs["bih_f"], np.float32),
                    np.asarray(inputs["bih_b"], np.float32)])
          + np.stack([np.asarray(inputs["bhh_f"], np.float32),
                      np.asarray(inputs["bhh_b"], np.float32)]))
    lbp = lb[:, PERM4H].reshape(2, 8, 128)      # (dir, m, p)
    d["lbias"] = np.ascontiguousarray(np.transpose(lbp, (2, 0, 1)))
    return d


_NC_CACHE = {}


def kernel(**inputs):
    key = "debug" if inputs.pop("_debug", False) else "main"
    if key not in _NC_CACHE:
        _NC_CACHE[key] = build(debug=(key == "debug"))
    nc = _NC_CACHE[key]
    in_maps = [prep_inputs(inputs, c) for c in range(NCORES)]
    res = bass_utils.run_bass_kernel_spmd(nc, in_maps,
                                          core_ids=list(range(NCORES)))
    out = np.concatenate([res.results[c]["out"] for c in range(NCORES)], axis=0)
    kernel.last_results = res
    return out


# revision 32
# speedup vs baseline: 1.8006x; 1.0253x over previous
"""CaptchaCRNN Trainium2 kernel: 7 convs + 2 train-mode BN + maxpools + biLSTM.

Data-parallel over batch on 8 NeuronCores (8 images/core). BN batch stats are
globalized with a tiny AllReduce. Conv matmuls run in float32r (1 cyc/row).
"""
import sys

sys.path.insert(0, "/opt/trn_rl_repo")

import numpy as np
import concourse.bass as bass
import concourse.bacc as bacc
import concourse.tile as tile
from concourse import masks
from concourse import mybir
from concourse import bass_utils

F32 = mybir.dt.float32
F16 = mybir.dt.float16
F32R = mybir.dt.float32r
AF = mybir.ActivationFunctionType
ALU = mybir.AluOpType
AX = mybir.AxisListType

NCORES = 8
B = 8          # images per core
EPS = 1e-5
INV_N = 1.0 / (64 * 8 * 32)   # BN normalizer: full batch 64 x H8 x W32

# 4H gate permutation: torch order [i,f,g,o] -> compute order [i,f,o,g]
PERM4H = np.r_[0:512, 768:1024, 512:768]


def _ap(obj, offset, dims):
    base = obj if isinstance(obj, bass.AP) else obj[:]
    return bass.AP(tensor=base.tensor, offset=base.offset + offset,
                   ap=[list(d) for d in dims])


def build(debug=False):
    nc = bacc.Bacc("TRN2", target_bir_lowering=False, debug=False,
                   enable_asserts=True, num_devices=NCORES)

    def din(name, shape):
        return nc.dram_tensor(name, list(shape), F32, kind="ExternalInput").ap()

    def dout(name, shape):
        return nc.dram_tensor(name, list(shape), F32, kind="ExternalOutput").ap()

    xim = din("xim", (B, 9, 64, 256))
    w1T = din("w1T", (9, 64))
    b1 = din("b1", (64, 1))
    w2p = din("w2p", (3, 128, 128))
    w2s = din("w2s", (3, 64, 128))
    w3T = din("w3T", (1, 9, 128, 256))
    w4T = din("w4T", (2, 9, 128, 256))
    w5T = din("w5T", (2, 9, 128, 512))
    w6T = din("w6T", (4, 9, 128, 512))
    w7T = din("w7T", (4, 4, 128, 512))
    b2 = din("b2", (128, 1))
    b3 = din("b3", (128, 2))
    b4 = din("b4", (128, 2))
    b5 = din("b5", (128, 4))
    b6 = din("b6", (128, 4))
    b7 = din("b7", (128, 4))
    gam = din("gam", (128, 4))
    bet = din("bet", (128, 4))
    wihT = din("wihT", (2, 8, 128, 1024))
    whhT = nc.dram_tensor("whhT", [2, 2, 128, 1024], mybir.dt.float16,
                          kind="ExternalInput").ap()
    lbias = din("lbias", (128, 2, 8))
    out = dout("out", (B, 15, 512))

    dbg = {}
    if debug:
        dbg["a2"] = dout("dbg_a2", (128, 8, 16, 64))
        dbg["a4"] = dout("dbg_a4", (128, 2, 8, 8, 32))
        dbg["a5"] = dout("dbg_a5", (128, 4, 8, 8, 32))
        dbg["c6p"] = dout("dbg_c6p", (128, 4, 8, 4, 16))
        dbg["c7"] = dout("dbg_c7", (128, 4, 8, 3, 16))
        dbg["xg"] = dout("dbg_xg", (128, 2, 8, 8, 15))
        dbg["hs"] = dout("dbg_hs", (128, 2, 2, 8, 15))

    with tile.TileContext(nc) as tc:
        opened = []

        def popen(name, bufs, space="SBUF", side=None):
            cm = tc.tile_pool(name=name, bufs=bufs, space=space, side=side)
            p = cm.__enter__()
            p._cm = cm
            opened.append(p)
            return p

        def pclose(p):
            p._cm.__exit__(None, None, None)
            opened.remove(p)

        const = popen("const", 1, side="left")
        psum = popen("psum", 8, space="PSUM")
        dram = popen("dram", 1, space="DRAM")

        # ---- constants ----
        b1_sb = const.tile([64, 1], F32)
        nc.sync.dma_start(out=b1_sb[:], in_=b1)
        b2_sb = const.tile([128, 1], F32)
        nc.sync.dma_start(out=b2_sb[:], in_=b2)
        b3_sb = const.tile([128, 2], F32)
        nc.sync.dma_start(out=b3_sb[:], in_=b3)
        b4_sb = const.tile([128, 2], F32)
        nc.sync.dma_start(out=b4_sb[:], in_=b4)
        b5_sb = const.tile([128, 4], F32)
        nc.sync.dma_start(out=b5_sb[:], in_=b5)
        b6_sb = const.tile([128, 4], F32)
        nc.sync.dma_start(out=b6_sb[:], in_=b6)
        b7_sb = const.tile([128, 4], F32)
        nc.sync.dma_start(out=b7_sb[:], in_=b7)
        gam_sb = const.tile([128, 4], F32)
        nc.sync.dma_start(out=gam_sb[:], in_=gam)
        bet_sb = const.tile([128, 4], F32)
        nc.sync.dma_start(out=bet_sb[:], in_=bet)
        eps_sb = const.tile([128, 1], F32)
        nc.vector.memset(eps_sb[:], EPS)
        ident = const.tile([128, 128], F32)
        masks.make_identity(nc, ident[:])

        # ---- conv1..4 weights ----
        wA = popen("wA", 1, side="left")
        w1_sb = wA.tile([9, 64], F32R)
        nc.sync.dma_start(out=w1_sb[:], in_=w1T.bitcast(F32R))
        w2p_sb = wA.tile([128, 3, 128], F32R)
        nc.sync.dma_start(
            out=w2p_sb[:],
            in_=_ap(w2p, 0, [[128, 128], [128 * 128, 3], [1, 128]]).bitcast(F32R))
        w2s_sb = wA.tile([128, 3, 128], F32R)
        nc.sync.dma_start(
            out=w2s_sb[64:128, :, :],
            in_=_ap(w2s, 0, [[128, 64], [64 * 128, 3], [1, 128]]).bitcast(F32R))
        w3_sb = wA.tile([128, 9, 256], F32R)
        nc.sync.dma_start(
            out=w3_sb[:],
            in_=_ap(w3T, 0, [[256, 128], [128 * 256, 9], [1, 256]]).bitcast(F32R))
        w4_sb = wA.tile([128, 2, 9, 256], F32R)
        nc.sync.dma_start(
            out=w4_sb[:],
            in_=_ap(w4T, 0, [[256, 128], [9 * 128 * 256, 2], [128 * 256, 9],
                             [1, 256]]).bitcast(F32R))

        # ---- conv1 + pool1 -> p1d DRAM (B, 64, 32, 128) ----
        p1d = dram.tile([B, 64, 32, 128], F32)
        rhs1 = popen("rhs1", 2, side="left")
        st1 = popen("st1", 2, side="left")
        for b in range(B):
            for q in range(4):
                r0 = 16 * q
                rt = rhs1.tile([9, 16, 256], F32R, tag="rhs1")
                nc.sync.dma_start(
                    out=rt[:],
                    in_=_ap(xim, b * (9 * 64 * 256) + r0 * 256,
                            [[64 * 256, 9], [1, 16 * 256]]).bitcast(F32R))
                s1q = st1.tile([64, 16, 256], F32, tag="s1q")
                rv = rt[:].rearrange("k (j t) w -> k j (t w)", t=2)
                for j in range(8):
                    pt = psum.tile([64, 512], F32, tag="ps")
                    nc.tensor.matmul(pt[:], w1_sb[:], rv[:, j, :],
                                     start=True, stop=True)
                    nc.scalar.activation(
                        out=s1q[:, 2 * j:2 * j + 2, :].rearrange("k a b -> k (a b)"),
                        in_=pt[:], func=AF.Identity, bias=b1_sb[:, 0:1], scale=1.0)
                wq = st1.tile([64, 16, 128], F32, tag="wq")
                s1v = s1q[:].rearrange("k h (w two) -> k h w two", two=2)
                nc.vector.tensor_max(out=wq[:], in0=s1v[:, :, :, 0],
                                     in1=s1v[:, :, :, 1])
                hq = st1.tile([64, 8, 128], F32, tag="hq")
                wv = wq[:].rearrange("k (h two) w -> k h two w", two=2)
                nc.vector.tensor_max(out=hq[:], in0=wv[:, :, 0, :],
                                     in1=wv[:, :, 1, :])
                nc.sync.dma_start(
                    out=_ap(p1d, b * (64 * 32 * 128) + (8 * q) * 128,
                            [[32 * 128, 64], [128, 8], [1, 128]]),
                    in_=hq[:])
        pclose(st1)
        pclose(rhs1)

        # ---- conv2 + pool2 -> a2p_t[b] [128, 18, 66] x8 ----
        a2p_pool = popen("a2p", 8, side="right")
        a2p_t = [a2p_pool.tile([128, 18, 66], F32R, tag="a2p", name=f"a2p{b}")
                 for b in range(B)]

        a1b_pool = popen("a1b", 2, side="left")
        st2 = popen("st2", 3, side="left")
        for b in range(B):
            a2p = a2p_t[b]
            for r in (0, 17):
                nc.gpsimd.memset(a2p[:, r, :].bitcast(F32), 0.0)
            for c in (0, 65):
                nc.gpsimd.memset(a2p[:, :, c].bitcast(F32), 0.0)
            a1b = a1b_pool.tile([128, 34, 130], F32R, tag="a1b")
            for r in (0, 33):
                nc.gpsimd.memset(a1b[0:64, r, :].bitcast(F32), 0.0)
            nc.gpsimd.memset(a1b[64:128, 32:34, :].bitcast(F32), 0.0)
            for c in (0, 129):
                nc.gpsimd.memset(a1b[:, :, c].bitcast(F32), 0.0)
            src = _ap(p1d, b * (64 * 32 * 128),
                      [[32 * 128, 64], [128, 32], [1, 128]]).bitcast(F32R)
            nc.gpsimd.dma_start(out=a1b[0:64, 1:33, 1:129], in_=src)
            nc.gpsimd.dma_start(out=a1b[64:128, 0:32, 1:129], in_=src)
            for n in range(8):
                h0 = 4 * n
                pt = psum.tile([128, 4, 128], F32, tag="ps")
                for kw in range(3):
                    nc.tensor.matmul(
                        pt[:], w2p_sb[:, kw, :],
                        a1b[0:128, h0:h0 + 4, kw:kw + 128],
                        start=(kw == 0), stop=False)
                for kw in range(3):
                    nc.tensor.matmul(
                        pt[:], w2s_sb[64:128, kw, :],
                        a1b[64:128, h0 + 1:h0 + 5, kw:kw + 128],
                        start=False, stop=(kw == 2))
                s2 = st2.tile([128, 4, 128], F32, tag="s2")
                nc.scalar.copy(out=s2[:].rearrange("p a b -> p (a b)"), in_=pt[:])
                w2m = st2.tile([128, 4, 64], F32, tag="w2m")
                s2v = s2[:].rearrange("p h (w two) -> p h w two", two=2)
                nc.vector.tensor_max(out=w2m[:], in0=s2v[:, :, :, 0],
                                     in1=s2v[:, :, :, 1])
                wv = w2m[:].rearrange("p (h two) w -> p h two w", two=2)
                nc.vector.tensor_max(out=a2p[:, 1 + 2 * n:3 + 2 * n, 1:65],
                                     in0=wv[:, :, 0, :], in1=wv[:, :, 1, :])
            nc.scalar.activation(out=a2p[:, 1:17, 1:65],
                                 in_=a2p[:, 1:17, 1:65].bitcast(F32),
                                 func=AF.Identity, bias=b2_sb[:, 0:1], scale=1.0)
        pclose(st2)
        pclose(a1b_pool)
        if debug:
            for b in range(B):
                nc.sync.dma_start(
                    out=_ap(dbg["a2"], b * 1024, [[8192, 128], [64, 16], [1, 64]]),
                    in_=a2p_t[b][:, 1:17, 1:65].bitcast(F32))

        # ---- conv3 -> a3_t[g] [128, B, 18, 66] x2 ----
        a3_pool = popen("a3", 2, side="left")
        a3_t = []
        for g in range(2):
            t = a3_pool.tile([128, B, 18, 66], F32R, tag="a3", name=f"a3_{g}")
            a3_t.append(t)
            for r in (0, 17):
                nc.gpsimd.memset(t[:, :, r, :].bitcast(F32), 0.0)
            for c in (0, 65):
                nc.gpsimd.memset(t[:, :, :, c].bitcast(F32), 0.0)
        for b in range(B):
            for m in range(2):
                for n in range(2):
                    pt = psum.tile([128, 8, 64], F32, tag="ps")
                    for kh in range(3):
                        for kw in range(3):
                            tap = kh * 3 + kw
                            nc.tensor.matmul(
                                pt[:], w3_sb[:, tap, 128 * m:128 * m + 128],
                                a2p_t[b][:, 8 * n + kh:8 * n + kh + 8, kw:kw + 64],
                                start=(tap == 0), stop=(tap == 8))
                    nc.scalar.activation(
                        out=a3_t[m][:, b, 1 + 8 * n:9 + 8 * n, 1:65],
                        in_=pt[:], func=AF.Identity, bias=b3_sb[:, m:m + 1],
                        scale=1.0)
        pclose(a2p_pool)

        # ---- conv4 + pool4 -> a4p_t[g] [128, B, 10, 34] x2 ----
        a4p_pool = popen("a4p", 2, side="right")
        a4p_t = []
        for g in range(2):
            t = a4p_pool.tile([128, B, 10, 34], F32R, tag="a4p", name=f"a4p{g}")
            a4p_t.append(t)
            for r in (0, 9):
                nc.gpsimd.memset(t[:, :, r, :].bitcast(F32), 0.0)
            for c in (0, 33):
                nc.gpsimd.memset(t[:, :, :, c].bitcast(F32), 0.0)
        # prefetch w5
        wB = popen("wB", 1, side="right")
        w5_sb = wB.tile([128, 2, 9, 512], F32R)
        nc.gpsimd.dma_start(
            out=w5_sb[:],
            in_=_ap(w5T, 0, [[512, 128], [9 * 128 * 512, 2], [128 * 512, 9],
                             [1, 512]]).bitcast(F32R))
        st4 = popen("st4", 3, side="right")
        for b in range(B):
            for m in range(2):
                for n in range(2):
                    pt = psum.tile([128, 8, 64], F32, tag="ps")
                    for g in range(2):
                        for kh in range(3):
                            for kw in range(3):
                                i = g * 9 + kh * 3 + kw
                                nc.tensor.matmul(
                                    pt[:], w4_sb[:, g, kh * 3 + kw,
                                                 128 * m:128 * m + 128],
                                    a3_t[g][:, b, 8 * n + kh:8 * n + kh + 8,
                                            kw:kw + 64],
                                    start=(i == 0), stop=(i == 17))
                    s4 = st4.tile([128, 8, 64], F32, tag="s4")
                    nc.scalar.copy(out=s4[:].rearrange("p a b -> p (a b)"),
                                   in_=pt[:])
                    s4v = s4[:].rearrange("p (h two) (w v) -> p h two w v",
                                          two=2, v=2)
                    nc.vector.tensor_max(
                        out=a4p_t[m][:, b, 1 + 4 * n:5 + 4 * n, 1:33],
                        in0=s4v[:, :, 0, :, 0], in1=s4v[:, :, 0, :, 1])
        for m in range(2):
            nc.scalar.activation(out=a4p_t[m][:, :, 1:9, 1:33],
                                 in_=a4p_t[m][:, :, 1:9, 1:33].bitcast(F32),
                                 func=AF.Identity, bias=b4_sb[:, m:m + 1],
                                 scale=1.0)
        pclose(st4)
        pclose(a3_pool)
        pclose(wA)
        if debug:
            for m in range(2):
                for b in range(B):
                    nc.sync.dma_start(
                        out=_ap(dbg["a4"], m * 2048 + b * 256,
                                [[4096, 128], [32, 8], [1, 32]]),
                        in_=a4p_t[m][:, b, 1:9, 1:33].bitcast(F32))

        # ---- LSTM weights part 1 (whh, lbias, wih dir0) — loads during conv5 ----
        wE1 = popen("wE1", 1, side="left")
        whh_sb = wE1.tile([128, 2, 2, 1024], F16)
        nc.gpsimd.dma_start(
            out=whh_sb[:],
            in_=_ap(whhT, 0, [[1024, 128], [2 * 128 * 1024, 2], [128 * 1024, 2],
                              [1, 1024]]))
        lb_sb = wE1.tile([128, 2, 8], F32)
        nc.sync.dma_start(out=lb_sb[:], in_=lbias)
        wih0_sb = wE1.tile([128, 8, 1024], F32)
        nc.gpsimd.dma_start(
            out=wih0_sb[:],
            in_=_ap(wihT, 0, [[1024, 128], [128 * 1024, 8], [1, 1024]]))

        # ---- w6 streaming (2 waves x 4 g-chunks, 2 prefetched) ----
        wC = popen("wC", 2, side="left")
        w6seq = [(wv, g) for wv in range(2) for g in range(4)]
        w6tiles = {}

        def load_w6(i):
            wv, g = w6seq[i]
            t = wC.tile([128, 9, 512], F32R, tag="w6g", name=f"w6g_{wv}_{g}")
            nc.gpsimd.dma_start(
                out=t[:],
                in_=_ap(w6T, g * (9 * 128 * 512),
                        [[512, 128], [128 * 512, 9], [1, 512]]).bitcast(F32R))
            w6tiles[(wv, g)] = t

        load_w6(0)
        load_w6(1)

        # ---- conv5 -> c5_t[m] x4; BN5 -> a5_t[g] x4 ----
        stat_pool = popen("stat", 1, side="left")
        scr_pool = popen("scr", 1, side="left")
        c5_pool = popen("c5", 4, side="left")
        c5_t = [c5_pool.tile([128, B, 8, 32], F32, tag="c5", name=f"c5_{m}")
                for m in range(4)]
        st5 = stat_pool.tile([128, 8], F32)
        for bp in range(4):
            b0 = 2 * bp
            for m in range(4):
                pt = psum.tile([128, 2, 8, 32], F32, tag="ps")
                for g in range(2):
                    for kh in range(3):
                        for kw in range(3):
                            i = g * 9 + kh * 3 + kw
                            nc.tensor.matmul(
                                pt[:], w5_sb[:, g, kh * 3 + kw,
                                             128 * m:128 * m + 128],
                                a4p_t[g][:, b0:b0 + 2, kh:kh + 8, kw:kw + 32],
                                start=(i == 0), stop=(i == 17))
                nc.scalar.activation(
                    out=c5_t[m][:, b0:b0 + 2, :, :], in_=pt[:],
                    func=AF.Identity, bias=b5_sb[:, m:m + 1], scale=1.0)
        pclose(wB)
        pclose(a4p_pool)

        # BN5 stats (per m, overlapping conv5 tail)
        for m in range(4):
            nc.vector.tensor_reduce(
                out=st5[:, m:m + 1],
                in_=c5_t[m][:].rearrange("p b h w -> p (b h w)"),
                axis=AX.X, op=ALU.add)
            scr = scr_pool.tile([128, 2048], F32, tag="scr")
            nc.scalar.activation(
                out=scr[:], in_=c5_t[m][:].rearrange("p b h w -> p (b h w)"),
                func=AF.Square, bias=0.0, scale=1.0,
                accum_out=st5[:, 4 + m:5 + m])
        cc5i = dram.tile([128, 8], F32, tag="cc5i")
        cc5o = dram.tile([128, 8], F32, tag="cc5o")
        nc.gpsimd.dma_start(out=cc5i[:], in_=st5[:])
        nc.gpsimd.collective_compute(
            "AllReduce", ALU.add, replica_groups=[list(range(NCORES))],
            ins=[cc5i[:].opt()], outs=[cc5o[:].opt()])
        g5 = stat_pool.tile([128, 8], F32)
        nc.sync.dma_start(out=g5[:], in_=cc5o[:])
        ms5 = stat_pool.tile([128, 8], F32)
        nc.scalar.activation(out=ms5[:], in_=g5[:], func=AF.Copy,
                             bias=0.0, scale=INV_N)
        var5 = stat_pool.tile([128, 4], F32)
        nc.vector.tensor_mul(out=var5[:], in0=ms5[:, 0:4], in1=ms5[:, 0:4])
        nc.vector.tensor_sub(out=var5[:], in0=ms5[:, 4:8], in1=var5[:])
        std5 = stat_pool.tile([128, 4], F32)
        nc.scalar.activation(out=std5[:], in_=var5[:], func=AF.Sqrt,
                             bias=eps_sb[:, 0:1], scale=1.0)
        nc.vector.reciprocal(out=std5[:], in_=std5[:])
        aa5 = stat_pool.tile([128, 4], F32)
        nc.vector.tensor_mul(out=aa5[:], in0=std5[:], in1=gam_sb[:])
        dd5 = stat_pool.tile([128, 4], F32)
        nc.vector.tensor_mul(out=dd5[:], in0=ms5[:, 0:4], in1=aa5[:])
        nc.vector.tensor_sub(out=dd5[:], in0=bet_sb[:], in1=dd5[:])

        a5_pool = popen("a5", 4, side="right")
        a5_t = []
        for g in range(4):
            t = a5_pool.tile([128, B, 10, 34], F32R, tag="a5", name=f"a5_{g}")
            a5_t.append(t)
            for r in (0, 9):
                nc.gpsimd.memset(t[:, :, r, :].bitcast(F32), 0.0)
            for c in (0, 33):
                nc.gpsimd.memset(t[:, :, :, c].bitcast(F32), 0.0)
        for m in range(4):
            nc.scalar.activation(
                out=a5_t[m][:, :, 1:9, 1:33], in_=c5_t[m][:],
                func=AF.Identity, bias=dd5[:, m:m + 1], scale=aa5[:, m:m + 1])
        pclose(c5_pool)
        if debug:
            for m in range(4):
                for b in range(B):
                    nc.sync.dma_start(
                        out=_ap(dbg["a5"], m * 2048 + b * 256,
                                [[8192, 128], [32, 8], [1, 32]]),
                        in_=a5_t[m][:, b, 1:9, 1:33].bitcast(F32))

        # ---- conv6 -> c6_t[m] x4; stats6; pool6 -> c6p_t[g]; BN6 on pooled ----
        c6_pool = popen("c6", 4, side="left")
        c6_t = [c6_pool.tile([128, B, 8, 32], F32, tag="c6", name=f"c6_{m}")
                for m in range(4)]
        st6 = stat_pool.tile([128, 8], F32)
        for wave in range(2):
            bps = [2 * wave, 2 * wave + 1]
            pts = {}
            for m in range(4):
                for bp in bps:
                    pts[(m, bp)] = psum.tile([128, 2, 8, 32], F32, tag="ps",
                                             name=f"ps6_{m}_{bp}")
            for g in range(4):
                i = wave * 4 + g
                w6g = w6tiles.pop((wave, g))
                if i + 2 < len(w6seq):
                    load_w6(i + 2)
                for tap in range(9):
                    kh, kw = tap // 3, tap % 3
                    for m in range(4):
                        for bp in bps:
                            b0 = 2 * bp
                            ii = g * 9 + tap
                            nc.tensor.matmul(
                                pts[(m, bp)][:],
                                w6g[:, tap, 128 * m:128 * m + 128],
                                a5_t[g][:, b0:b0 + 2, kh:kh + 8, kw:kw + 32],
                                start=(ii == 0), stop=(ii == 35))
            for m in range(4):
                for bp in bps:
                    b0 = 2 * bp
                    nc.scalar.activation(
                        out=c6_t[m][:, b0:b0 + 2, :, :], in_=pts[(m, bp)][:],
                        func=AF.Identity, bias=b6_sb[:, m:m + 1], scale=1.0)
        pclose(a5_pool)

        # stats6 per m
        for m in range(4):
            nc.vector.tensor_reduce(
                out=st6[:, m:m + 1],
                in_=c6_t[m][:].rearrange("p b h w -> p (b h w)"),
                axis=AX.X, op=ALU.add)
            scr = scr_pool.tile([128, 2048], F32, tag="scr")
            nc.scalar.activation(
                out=scr[:], in_=c6_t[m][:].rearrange("p b h w -> p (b h w)"),
                func=AF.Square, bias=0.0, scale=1.0,
                accum_out=st6[:, 4 + m:5 + m])
        cc6i = dram.tile([128, 8], F32, tag="cc6i")
        cc6o = dram.tile([128, 8], F32, tag="cc6o")
        nc.gpsimd.dma_start(out=cc6i[:], in_=st6[:])
        nc.gpsimd.collective_compute(
            "AllReduce", ALU.add, replica_groups=[list(range(NCORES))],
            ins=[cc6i[:].opt()], outs=[cc6o[:].opt()])

        # pool6 (independent of stats) -> c6p_t[g] [128, B, 4, 17]
        c6p_pool = popen("c6p", 4, side="right")
        wD = popen("wD", 1, side="right")
        w7_sb = wD.tile([128, 4, 4, 512], F32R)
        nc.gpsimd.dma_start(
            out=w7_sb[:],
            in_=_ap(w7T, 0, [[512, 128], [4 * 128 * 512, 4], [128 * 512, 4],
                             [1, 512]]).bitcast(F32R))
        c6p_t = []
        for g in range(4):
            t = c6p_pool.tile([128, B, 4, 17], F32R, tag="c6p", name=f"c6p{g}")
            c6p_t.append(t)
            nc.gpsimd.memset(t[:, :, :, 16].bitcast(F32), 0.0)
        for m in range(4):
            c6v = c6_t[m][:].rearrange("p b (h two) (w v) -> p b h two w v",
                                       two=2, v=2)
            nc.vector.tensor_max(out=c6p_t[m][:, :, :, 0:16],
                                 in0=c6v[:, :, :, 0, :, 0],
                                 in1=c6v[:, :, :, 0, :, 1])

        g6 = stat_pool.tile([128, 8], F32)
        nc.sync.dma_start(out=g6[:], in_=cc6o[:])
        ms6 = stat_pool.tile([128, 8], F32)
        nc.scalar.activation(out=ms6[:], in_=g6[:], func=AF.Copy,
                             bias=0.0, scale=INV_N)
        var6 = stat_pool.tile([128, 4], F32)
        nc.vector.tensor_mul(out=var6[:], in0=ms6[:, 0:4], in1=ms6[:, 0:4])
        nc.vector.tensor_sub(out=var6[:], in0=ms6[:, 4:8], in1=var6[:])
        std6 = stat_pool.tile([128, 4], F32)
        nc.scalar.activation(out=std6[:], in_=var6[:], func=AF.Sqrt,
                             bias=eps_sb[:, 0:1], scale=1.0)
        nc.vector.reciprocal(out=std6[:], in_=std6[:])
        aa6 = stat_pool.tile([128, 4], F32)
        nc.vector.tensor_mul(out=aa6[:], in0=std6[:], in1=gam_sb[:])
        dd6 = stat_pool.tile([128, 4], F32)
        nc.vector.tensor_mul(out=dd6[:], in0=ms6[:, 0:4], in1=aa6[:])
        nc.vector.tensor_sub(out=dd6[:], in0=bet_sb[:], in1=dd6[:])
        for m in range(4):
            nc.scalar.activation(
                out=c6p_t[m][:], in_=c6p_t[m][:].bitcast(F32),
                func=AF.Identity, bias=dd6[:, m:m + 1], scale=aa6[:, m:m + 1])
        pclose(c6_pool)
        pclose(scr_pool)
        pclose(stat_pool)
        pclose(wC)
        if debug:
            for g in range(4):
                for b in range(B):
                    nc.sync.dma_start(
                        out=_ap(dbg["c6p"], g * 512 + b * 64,
                                [[2048, 128], [16, 4], [1, 16]]),
                        in_=c6p_t[g][:, b, :, 0:16].bitcast(F32))

        # ---- LSTM weights part 2 (wih dir1) — loads during conv7/xproj-dir0 ----
        wE2 = popen("wE2", 1, side="left")
        wih1_sb = wE2.tile([128, 8, 1024], F32)
        nc.gpsimd.dma_start(
            out=wih1_sb[:],
            in_=_ap(wihT, 8 * 128 * 1024,
                    [[1024, 128], [128 * 1024, 8], [1, 1024]]))

        # ---- conv7 (VALID 2x2) -> c7 [128, 4, B, 3, 16] ----
        c7_pool = popen("c7", 1, side="left")
        c7 = c7_pool.tile([128, 4, B, 3, 16], F32)
        for m in range(4):
            pt = psum.tile([128, 8, 3, 16], F32, tag="ps")
            for g in range(4):
                for tap in range(4):
                    kh, kw = tap // 2, tap % 2
                    i = g * 4 + tap
                    nc.tensor.matmul(
                        pt[:], w7_sb[:, g, tap, 128 * m:128 * m + 128],
                        c6p_t[g][:, :, kh:kh + 3, kw:kw + 16],
                        start=(i == 0), stop=(i == 15))
            nc.scalar.activation(out=c7[:, m], in_=pt[:], func=AF.Identity,
                                 bias=b7_sb[:, m:m + 1], scale=1.0)
        pclose(wD)
        pclose(c6p_pool)
        if debug:
            nc.sync.dma_start(
                out=_ap(dbg["c7"], 0, [[1536, 128], [1, 1536]]),
                in_=c7[:].rearrange("p g b h w -> p (g b h w)"))

        # ---- LSTM ----
        ls = popen("ls", 1, side="right")
        xg = ls.tile([128, 2, 8, 8, 15], F32)
        hs = ls.tile([128, 2, 2, 8, 15], F16)
        hs32 = ls.tile([128, 2, 2, 8, 15], F32)
        cst = ls.tile([128, 2, 2, 8], F32)
        gp = popen("gp", 4, side="right")
        tp = popen("tp", 4, side="right")

        for dr in range(2):
            wih_d = wih0_sb if dr == 0 else wih1_sb
            for m in range(8):
                pt = psum.tile([128, 8, 16], F32, tag="ps")
                for gd in range(8):
                    nc.tensor.matmul(
                        pt[:], wih_d[:, gd, 128 * m:128 * m + 128],
                        c7[:, gd % 4, :, gd // 4, :],
                        start=(gd == 0), stop=(gd == 7))
                nc.scalar.activation(
                    out=xg[:, dr, m],
                    in_=pt[:, :, 0:15],
                    func=AF.Identity, bias=lb_sb[:, dr, m:m + 1], scale=1.0)
        if debug:
            nc.sync.dma_start(
                out=_ap(dbg["xg"], 0, [[1920, 128], [1, 1920]]),
                in_=xg[:].rearrange("p d m b t -> p (d m b t)"))

        for t in range(15):
            for dr in range(2):
                tt = t if dr == 0 else 14 - t
                ga = gp.tile([128, 8, 8], F32, tag="ga")
                if t == 0:
                    nc.vector.tensor_copy(out=ga[:], in_=xg[:, dr, :, :, tt])
                else:
                    tprev = tt - 1 if dr == 0 else tt + 1
                    pr = psum.tile([128, 8, 8], F32, tag="ps")
                    for m in range(8):
                        for gh in range(2):
                            nc.tensor.matmul(
                                pr[:, m, :],
                                whh_sb[:, dr, gh, 128 * m:128 * m + 128],
                                hs[:, dr, gh, :, tprev],
                                start=(gh == 0), stop=(gh == 1),
                                skip_group_check=True)
                    nc.vector.tensor_add(out=ga[:], in0=pr[:],
                                         in1=xg[:, dr, :, :, tt])
                nc.scalar.activation(out=ga[:, 0:6, :], in_=ga[:, 0:6, :],
                                     func=AF.Sigmoid, bias=0.0, scale=1.0)
                nc.scalar.activation(out=ga[:, 6:8, :], in_=ga[:, 6:8, :],
                                     func=AF.Tanh, bias=0.0, scale=1.0)
                cs = cst[:, dr]
                if t == 0:
                    nc.vector.tensor_mul(out=cs, in0=ga[:, 0:2, :],
                                         in1=ga[:, 6:8, :])
                else:
                    t1 = tp.tile([128, 2, 8], F32, tag="t1")
                    nc.vector.tensor_mul(out=t1[:], in0=ga[:, 0:2, :],
                                         in1=ga[:, 6:8, :])
                    t2 = tp.tile([128, 2, 8], F32, tag="t2")
                    nc.vector.tensor_mul(out=t2[:], in0=ga[:, 2:4, :], in1=cs)
                    nc.vector.tensor_add(out=cs, in0=t1[:], in1=t2[:])
                th = tp.tile([128, 2, 8], F32, tag="th")
                nc.scalar.activation(out=th[:], in_=cs, func=AF.Tanh,
                                     bias=0.0, scale=1.0)
                nc.vector.tensor_mul(out=hs[:, dr, :, :, tt],
                                     in0=ga[:, 4:6, :], in1=th[:])
                nc.vector.tensor_mul(out=hs32[:, dr, :, :, tt],
                                     in0=ga[:, 4:6, :], in1=th[:])

        if debug:
            nc.sync.dma_start(
                out=_ap(dbg["hs"], 0, [[480, 128], [1, 480]]),
                in_=hs32[:].rearrange("p d g b t -> p (d g b t)"))

        # ---- output: PE-transpose h to (b*t)-partitions, then contiguous DMA ----
        ptt = psum.tile([120, 512], F32, tag="ps")
        for dr in range(2):
            for gh in range(2):
                nc.tensor.transpose(
                    ptt[:, 128 * (2 * dr + gh):128 * (2 * dr + gh) + 128],
                    hs32[:, dr, gh].rearrange("p b t -> p (b t)"),
                    ident[:])
        outsb = ls.tile([120, 512], F32)
        nc.scalar.copy(out=outsb[:], in_=ptt[:])
        nc.sync.dma_start(out=_ap(out, 0, [[512, 120], [1, 512]]), in_=outsb[:])

        for p in reversed(list(opened)):
            pclose(p)

    nc.compile()
    return nc


def prep_inputs(inputs, core):
    """Host-side: shard + transform weights for one core."""
    d = {}
    x = np.asarray(inputs["x"], dtype=np.float32)
    xs = x[core * B:(core + 1) * B, 0]          # (8, 64, 256)
    xp = np.zeros((B, 66, 258), np.float32)
    xp[:, 1:65, 1:257] = xs
    xim = np.empty((B, 9, 64, 256), np.float32)
    for kh in range(3):
        for kw in range(3):
            xim[:, kh * 3 + kw] = xp[:, kh:kh + 64, kw:kw + 256]
    d["xim"] = xim

    w1 = np.asarray(inputs["w1"], np.float32)   # (64,1,3,3)
    d["w1T"] = np.ascontiguousarray(
        w1[:, 0].reshape(64, 9).T)              # (9, 64)
    d["b1"] = np.asarray(inputs["b1"], np.float32).reshape(64, 1)

    w2 = np.asarray(inputs["w2"], np.float32)   # (128,64,3,3)
    w2p = np.zeros((3, 128, 128), np.float32)
    for kw in range(3):
        w2p[kw, 0:64] = w2[:, :, 0, kw].T
        w2p[kw, 64:128] = w2[:, :, 1, kw].T
    d["w2p"] = w2p
    d["w2s"] = np.ascontiguousarray(
        np.transpose(w2[:, :, 2, :], (2, 1, 0)))  # (3, 64, 128)
    d["b2"] = np.asarray(inputs["b2"], np.float32).reshape(1, 128).T.copy()

    def wT(w, gK, cout):
        # w (O, I, 3, 3) -> (gK, 9, 128, O)
        o, i_, kh, kw = w.shape
        r = np.transpose(w, (2, 3, 1, 0)).reshape(kh * kw, gK, 128, o)
        return np.ascontiguousarray(np.transpose(r, (1, 0, 2, 3)))

    d["w3T"] = wT(np.asarray(inputs["w3"], np.float32), 1, 256)
    d["w4T"] = wT(np.asarray(inputs["w4"], np.float32), 2, 256)
    d["w5T"] = wT(np.asarray(inputs["w5"], np.float32), 2, 512)
    d["w6T"] = wT(np.asarray(inputs["w6"], np.float32), 4, 512)
    w7 = np.asarray(inputs["w7"], np.float32)   # (512,512,2,2)
    r7 = np.transpose(w7, (2, 3, 1, 0)).reshape(4, 4, 128, 512)
    d["w7T"] = np.ascontiguousarray(np.transpose(r7, (1, 0, 2, 3)))
    for k, g in (("b3", 2), ("b4", 2), ("b5", 4), ("b6", 4), ("b7", 4)):
        d[k] = np.ascontiguousarray(
            np.asarray(inputs[k], np.float32).reshape(g, 128).T)
    d["gam"] = np.ascontiguousarray(
        np.asarray(inputs["gamma"], np.float32).reshape(4, 128).T)
    d["bet"] = np.ascontiguousarray(
        np.asarray(inputs["beta"], np.float32).reshape(4, 128).T)

    # LSTM: d-column permutation dmap maps compute-chunk col 128*j+p to
    # reference D index 2*(128*(j%4)+p) + j//4
    j = np.arange(8)[:, None]
    p = np.arange(128)[None, :]
    dmap = (2 * (128 * (j % 4) + p) + j // 4).reshape(-1)
    wih = np.stack([np.asarray(inputs["Wih_f"], np.float32),
                    np.asarray(inputs["Wih_b"], np.float32)])
    whh = np.stack([np.asarray(inputs["Whh_f"], np.float32),
                    np.asarray(inputs["Whh_b"], np.float32)])
    wihp = wih[:, PERM4H][:, :, dmap]           # (2, 1024, 1024)
    d["wihT"] = np.ascontiguousarray(
        np.transpose(wihp, (0, 2, 1)).reshape(2, 8, 128, 1024))
    whhp = whh[:, PERM4H]                       # (2, 1024, 256)
    d["whhT"] = np.ascontiguousarray(
        np.transpose(whhp, (0, 2, 1)).reshape(2, 2, 128, 1024)).astype(np.float16)
    lb = (np.stack([np.asarray(inputs["bih_f"], np.float32),
                    np.asarray(inputs["bih_b"], np.float32)])
          + np.stack([np.asarray(inputs["bhh_f"], np.float32),
                      np.asarray(inputs["bhh_b"], np.float32)]))
    lbp = lb[:, PERM4H].reshape(2, 8, 128)      # (dir, m, p)
    d["lbias"] = np.ascontiguousarray(np.transpose(lbp, (2, 0, 1)))
    return d


_NC_CACHE = {}


def kernel(**inputs):
    key = "debug" if inputs.pop("_debug", False) else "main"
    if key not in _NC_CACHE:
        _NC_CACHE[key] = build(debug=(key == "debug"))
    nc = _NC_CACHE[key]
    in_maps = [prep_inputs(inputs, c) for c in range(NCORES)]
    res = bass_utils.run_bass_kernel_spmd(nc, in_maps,
                                          core_ids=list(range(NCORES)))
    out = np.concatenate([res.results[c]["out"] for c in range(NCORES)], axis=0)
    kernel.last_results = res
    return out
